# revision 33
# baseline (speedup 1.0000x reference)
"""MLA (multi-head latent attention) Trainium2 Bass kernel, 8-core SPMD.

Sharding: 2-way data parallel over batch x 4-way tensor parallel over heads.
Core c handles batch b = c // 4 and heads [hg*4, hg*4+4) with hg = c % 4.
Each core computes the full MLA forward for its batch/heads and the partial
out-projection (row-sharded W_o); the host sums the 4 partials per batch.

v2: fully fused per-chunk pipeline. For each 512-token chunk g:
  S1: q (folded W_dq@[W_uq|W_qr], direct from x), c_kv, krx chains
  S2: k up-proj + rope, v up-proj
  A(g): causal attention for query chunk g over key chunks 0..g
  O(g): out-projection, DMA'd straight from PSUM
Engine balance: PE does all matmuls; Scalar does exp + PSUM evictions;
Pool (gpsimd) does causal 0/1-mask muls + softmax-Z accumulation; Vector
does rope + reciprocal + final normalize mul. Scores are issued 2 tiles
ahead of the AV matmuls and each head's Z-normalize chain is deferred into
the next head's score stream so the PE never waits on exp.

Layouts on device (partition dim first):
  xT      [128, 16, S]   x[b].T, feature-on-partition
  qTg     [128, 4, 512]  per head: rows 0:64 content, 64:96/96:128 rope halves
  kT      [128, 4, S]    same row layout
  v       [128, 16, 512] [token%128, token//128, head*128+d]
  scoresT [128k, 512q]   PSUM; exp'd on Scalar -> et (fp16); Z via pool adds
                         + ones-matmul; normalize at attn eviction.
All matmul operands are fp16 (1 cyc/row at full PE clock).
"""
import sys

sys.path.insert(0, "/opt/trn_rl_repo")

import numpy as np

import concourse.bacc as bacc
import concourse.mybir as mybir
import concourse.tile as tile
from concourse import bass_utils

H_DIM = 2048
N_HEADS = 16
D_HEAD = 128
D_ROPE = 64
D_NOPE = 64
HALF = D_ROPE // 2          # 32
C_DIM = 512
ROPE_BASE = 10000.0
HPC = 4                     # heads per core
B = 2
S_FULL = 2048
KC = H_DIM // 128           # 16
CC = C_DIM // 128           # 4
SCALE = 1.0 / float(np.sqrt(D_HEAD))

f16 = mybir.dt.float16
f32 = mybir.dt.float32


def build_nc(S=S_FULL):
    TC = S // 512            # token chunks / query groups

    nc = bacc.Bacc("TRN2", target_bir_lowering=False, debug=False)

    d_xT = nc.dram_tensor("xT", [TC, 128, KC * 512], f16, kind="ExternalInput")
    d_wq = nc.dram_tensor("wq", [128, KC, 512], f16, kind="ExternalInput")
    d_wdkv = nc.dram_tensor("wdkv", [128, KC, 128], f16, kind="ExternalInput")
    d_cin = nc.dram_tensor("cin", [TC, 128, 512], f16, kind="Internal")
    d_cout = nc.dram_tensor("cout", [TC, 4, 128, 512], f16, kind="Internal")
    d_wkrx1 = nc.dram_tensor("wkrx1", [128, KC, HPC * HALF], f16, kind="ExternalInput")
    d_wkrx2 = nc.dram_tensor("wkrx2", [128, KC, HPC * HALF], f16, kind="ExternalInput")
    d_wuk = nc.dram_tensor("wuk", [128, CC, HPC * D_NOPE], f16, kind="ExternalInput")
    d_wuv = nc.dram_tensor("wuv", [128, CC, HPC * D_HEAD], f16, kind="ExternalInput")
    d_wo = nc.dram_tensor("wo", [128, HPC, H_DIM], f16, kind="ExternalInput")
    d_cos = nc.dram_tensor("cosA", [128, S], f16, kind="ExternalInput")
    d_sin = nc.dram_tensor("sinA", [128, S], f16, kind="ExternalInput")
    d_mask = nc.dram_tensor("masks", [128, 4, 512], f16, kind="ExternalInput")
    d_onec = nc.dram_tensor("onec", [128, 1], f16, kind="ExternalInput")
    d_oner = nc.dram_tensor("oner", [1, 128], f16, kind="ExternalInput")
    d_o = nc.dram_tensor("o", [S, H_DIM], f32, kind="ExternalOutput")

    import contextlib
    with tile.TileContext(nc) as tc:
        with contextlib.ExitStack() as stack:
            def pool(name, **kw):
                return stack.enter_context(tc.tile_pool(name=name, **kw))

            p_const = pool("const", bufs=1)
            p_w = pool("w", bufs=1, side="right")
            p_x = pool("xp", bufs=2, side="right")
            p_kT = pool("kT", bufs=1)
            p_v = pool("vp", bufs=1)
            p_qT = pool("qT", bufs=2)
            p_ckv = pool("ckv", bufs=2, side="right")
            p_krx = pool("krx", bufs=1, side="right")
            p_tmp = pool("tmp", bufs=1, side="right")
            p_et = pool("et", bufs=4)
            p_za = pool("za", bufs=2)
            p_rz = pool("rz", bufs=1)
            p_rb = pool("rb", bufs=1)
            p_att = pool("att", bufs=2)
            p_ot = pool("ot", bufs=2)
            p_psA = pool("psA", bufs=2, space="PSUM")
            p_po = pool("ps_po", bufs=2, space="PSUM")
            p_pz = pool("ps_z", bufs=1, space="PSUM")
            p_pb = pool("ps_b", bufs=1, space="PSUM")

            # ---- constants + weights, ordered by first use. x chunks go on
            # the scalar engine's DMA queue so they overlap the weight DMAs
            # (sync queue).
            # Queue plan: scalar queue carries ONLY x chunks (chunk 0 first,
            # nothing ahead of it); sync queue carries the weights in first-use
            # order; the gpsimd SWDGE queue carries the small / late tensors.
            wq = p_w.tile([128, KC, 512], f16, tag="wq")
            nc.sync.dma_start(wq[:], d_wq.ap())
            wdkv = p_w.tile([128, KC, 128], f16, tag="wdkv")
            nc.sync.dma_start(wdkv[:], d_wdkv.ap())
            wkrx1 = p_w.tile([128, KC, HPC * HALF], f16, tag="wkrx1")
            nc.sync.dma_start(wkrx1[:], d_wkrx1.ap())
            wkrx2 = p_w.tile([128, KC, HPC * HALF], f16, tag="wkrx2")
            nc.sync.dma_start(wkrx2[:], d_wkrx2.ap())
            wo = p_w.tile([128, HPC, H_DIM], f16, tag="wo")
            nc.sync.dma_start(wo[:], d_wo.ap())
            cosA = p_const.tile([128, S], f16, tag="cosA")
            nc.gpsimd.dma_start(cosA[:], d_cos.ap())
            sinA = p_const.tile([128, S], f16, tag="sinA")
            nc.gpsimd.dma_start(sinA[:], d_sin.ap())
            wuk = p_w.tile([128, CC, HPC * D_NOPE], f16, tag="wuk")
            nc.gpsimd.dma_start(wuk[:], d_wuk.ap())
            wuv = p_w.tile([128, CC, HPC * D_HEAD], f16, tag="wuv")
            nc.gpsimd.dma_start(wuv[:], d_wuv.ap())
            masks = p_const.tile([128, 4, 512], f16, tag="masks")
            nc.gpsimd.dma_start(masks[:], d_mask.ap())
            onech = p_const.tile([128, 1], f16, tag="onec")
            nc.gpsimd.dma_start(onech[:], d_onec.ap())
            oner = p_const.tile([1, 128], f16, tag="oner")
            nc.gpsimd.dma_start(oner[:], d_oner.ap())

            # persistent K/V for all chunks
            kT = p_kT.tile([128, HPC, S], f16, tag="kT")
            v_sb = p_v.tile([128, S // 128, 512], f16, tag="v")

            def rope_psum(x1ap, x2ap, dst, gs, dc):
                # x1/x2: [128 = 4h*32, 512] (psum or sbuf); writes rope rows
                t1 = p_tmp.tile([128, 512], f16, tag="t1")
                t2 = p_tmp.tile([128, 512], f16, tag="t2")
                o1 = p_tmp.tile([128, 512], f16, tag="o1")
                o2 = p_tmp.tile([128, 512], f16, tag="o2")
                nc.vector.tensor_mul(t1[:], x1ap, cosA[:, gs])
                nc.vector.tensor_mul(t2[:], x2ap, sinA[:, gs])
                nc.vector.tensor_sub(o1[:], t1[:], t2[:])
                nc.vector.tensor_mul(t1[:], x1ap, sinA[:, gs])
                nc.vector.tensor_mul(t2[:], x2ap, cosA[:, gs])
                nc.vector.tensor_add(o2[:], t1[:], t2[:])
                for h in range(HPC):
                    hs = slice(h * HALF, (h + 1) * HALF)
                    nc.scalar.copy(dst[64:96, h, dc], o1[hs, :])
                    nc.scalar.copy(dst[96:128, h, dc], o2[hs, :])

            def z_part1(za_v, za_p):
                # zr[q]: (za_v + za_p) then summed over the pair dim
                zm = p_rb.tile([128, 2, 512], f16, tag="zm")
                nc.vector.tensor_add(zm[:], za_v[:], za_p[:])
                zr = p_rb.tile([128, 512], f16, tag="zr")
                nc.vector.tensor_add(zr[:], zm[:, 0, :], zm[:, 1, :])
                pz = p_pz.tile([1, 512], f32, tag="pz")
                nc.tensor.matmul(pz[:], onech[:], zr[:], start=True,
                                 stop=True)
                r0 = p_rz.tile([1, 512], f32, tag="r0")
                nc.vector.reciprocal(r0[:], pz[:])
                rzc = p_rz.tile([1, 512], f16, tag="rzc")
                nc.vector.tensor_copy(rzc[:], r0[:])
                return rzc

            def z_part2(h, rzc, po, attn_t):
                pb = p_pb.tile([128, 512], f32, tag="pb")
                nc.tensor.matmul(pb[:], oner[:], rzc[:], start=True, stop=True)
                rb = p_rb.tile([128, 512], f16, tag="rb")
                nc.scalar.copy(rb[:], pb[:])
                nc.vector.tensor_mul(attn_t[:, h, :], po[:], rb[:])

            def emit_O(og, attn_t):
                # out-projection for chunk og; nck pairs share one 2-bank
                # PSUM tile, evicted with a single copy + single DMA
                for t4 in range(4):
                    tt = og * 4 + t4
                    for np2 in range(2):
                        pso = p_psA.tile([128, 2, 512], f32, tag="ps")
                        for j in range(2):
                            nck = 2 * np2 + j
                            for h in range(HPC):
                                nc.tensor.matmul(
                                    pso[:, j, :],
                                    attn_t[:, h, t4 * 128:(t4 + 1) * 128],
                                    wo[:, h, nck * 512:(nck + 1) * 512],
                                    start=(h == 0), stop=(h == HPC - 1),
                                )
                        ot = p_ot.tile([128, 2, 512], f32, tag="ot")
                        nc.vector.tensor_copy(ot[:], pso[:])
                        nc.sync.dma_start(
                            d_o.ap()[tt * 128:(tt + 1) * 128,
                                     np2 * 1024:(np2 + 1) * 1024],
                            ot[:])

            carryZ = None   # last head of previous chunk: (h, za_v, za_p, po, attn)
            carryO = None   # previous chunk's out-projection: (g, attn)

            def ckv_shard_ag(cg, xtile):
                # this core's 128-row block of c_kv for chunk cg -> DRAM ->
                # AllGather across the 4-core batch group (one chunk ahead)
                ps = p_psA.tile([128, 2, 512], f32, tag="ps")
                for k in range(KC):
                    nc.tensor.matmul(
                        ps[:, 0, :], wdkv[:, k, :], xtile[:, k, :],
                        start=(k == 0), stop=(k == KC - 1),
                    )
                cown = p_krx.tile([128, 512], f16, tag="cown")
                nc.scalar.copy(cown[:], ps[:, 0, :])
                nc.gpsimd.dma_start(d_cin.ap()[cg], cown[:])
                nc.gpsimd.collective_compute(
                    "AllGather", mybir.AluOpType.bypass,
                    replica_groups=[[0, 1, 2, 3], [4, 5, 6, 7]],
                    ins=[d_cin.ap()[cg]],
                    outs=[d_cout.ap()[cg]],
                )

            xts = {}

            for g in range(TC):
                gs = slice(g * 512, (g + 1) * 512)
                if g == 0:
                    xt0 = p_x.tile([128, KC, 512], f16, tag="xt")
                    xts[0] = xt0
                    nc.scalar.dma_start(xt0[:], d_xT.ap()[0])
                    ckv_shard_ag(0, xt0)
                xt = xts.pop(g)
                if g + 1 < TC:
                    xtn = p_x.tile([128, KC, 512], f16, tag="xt")
                    xts[g + 1] = xtn
                    nc.scalar.dma_start(xtn[:], d_xT.ap()[g + 1])

                # ---- S1a: q direct from x (folded weights) ----
                # The previous chunk's last-head Z chain is issued between the
                # first chains so the PE never waits on it.
                qTg = p_qT.tile([128, HPC, 512], f16, tag="qTg")
                qrope = None
                for mp in range(2):
                    ps = p_psA.tile([128, 2, 512], f32, tag="ps")
                    for j in range(2):
                        m = 2 * mp + j
                        for k in range(KC):
                            nc.tensor.matmul(
                                ps[:, j, :], wq[:, k, m * 128:(m + 1) * 128],
                                xt[:, k, :],
                                start=(k == 0), stop=(k == KC - 1),
                            )
                    if mp == 0:
                        for hh in range(4):
                            nc.scalar.copy(
                                qTg[0:64, hh, :],
                                ps[64 * (hh % 2):64 * (hh % 2) + 64, hh // 2, :])
                        if carryZ is not None:
                            ch, czav, czap, cpo, cattn = carryZ
                            crz = z_part1(czav, czap)
                            carryZ = (ch, crz, cpo, cattn)
                    else:
                        qrope = ps
                        if carryZ is not None:
                            ch, crz, cpo, cattn = carryZ
                            z_part2(ch, crz, cpo, cattn)
                            carryZ = None
                rope_psum(qrope[:, 0, :], qrope[:, 1, :], qTg, gs, slice(0, 512))

                # ---- S1b: read gathered c_kv; compute next chunk's shard
                ckvg = p_ckv.tile([128, CC, 512], f16, tag="ckvg")
                for m in range(CC):
                    nc.sync.dma_start(ckvg[:, m, :], d_cout.ap()[g][m])
                if g + 1 < TC:
                    ckv_shard_ag(g + 1, xts[g + 1])

                # ---- S1c: krx chains ----
                kx = p_krx.tile([128, 2, 512], f16, tag="kx")
                ps = p_psA.tile([128, 2, 512], f32, tag="ps")
                for j, w_sb in enumerate((wkrx1, wkrx2)):
                    for k in range(KC):
                        nc.tensor.matmul(
                            ps[:, j, :], w_sb[:, k, :], xt[:, k, :],
                            start=(k == 0), stop=(k == KC - 1),
                        )
                nc.scalar.copy(kx[:], ps[:])

                # ---- S2: k up-proj (content) + rope; v up-proj ----
                ps = p_psA.tile([128, 2, 512], f32, tag="ps")
                for m2 in range(2):
                    for k in range(CC):
                        nc.tensor.matmul(
                            ps[:, m2, :], wuk[:, k, m2 * 128:(m2 + 1) * 128],
                            ckvg[:, k, :], start=(k == 0), stop=(k == CC - 1),
                        )
                for hh in range(4):
                    nc.scalar.copy(
                        kT[0:64, hh, gs],
                        ps[64 * (hh % 2):64 * (hh % 2) + 64, hh // 2, :])
                rope_psum(kx[:, 0, :], kx[:, 1, :], kT, gs, gs)

                for tp in range(2):
                    ps = p_psA.tile([128, 2, 512], f32, tag="ps")
                    for j in range(2):
                        tt = 2 * tp + j
                        for k in range(CC):
                            nc.tensor.matmul(
                                ps[:, j, :], ckvg[:, k, tt * 128:(tt + 1) * 128],
                                wuv[:, k, :], start=(k == 0), stop=(k == CC - 1),
                            )
                    nc.scalar.copy(v_sb[:, g * 4 + 2 * tp:g * 4 + 2 * tp + 2, :],
                                   ps[:])

                # ---- O(g-1): previous chunk's out-projection ----
                if carryO is not None:
                    cog, cattn = carryO
                    emit_O(cog, cattn)
                    carryO = None

                # ---- A(g): attention for query chunk g ----
                # kt order: diagonal (masked) tiles first so their longer
                # exp->mask chain hides under the unmasked tiles' stream.
                # Z accumulates in two tiles (vector even / pool odd) to halve
                # the serial add-chain; the normalize chain of head h is
                # issued inside head h+1's score stream.
                attn_g = p_att.tile([128, HPC, 512], f16, tag="attn")
                nkt = 4 * (g + 1)
                nktp = nkt // 2
                LOOKP = 2
                # kt pairs, diagonal (masked) pairs first
                ktp_order = ([(4 * g, 0), (4 * g + 2, 2)]
                             + [(2 * i, -1) for i in range(2 * g)])
                zchain = None

                zs1 = min(2, nktp)
                zs2 = min(4, nktp + LOOKP - 1)
                for h in range(HPC):
                    po = p_po.tile([128, 512], f32, tag="po")
                    za_v = za_p = None
                    if nktp > 2:
                        za_v = p_za.tile([128, 2, 512], f32, tag="za_v")
                        za_p = p_za.tile([128, 2, 512], f32, tag="za_p")
                    ets = {}
                    zinit = {}
                    for step in range(nktp + LOOKP):
                        if step < nktp:
                            kt0, d = ktp_order[step]
                            ps = p_psA.tile([128, 2, 512], f32, tag="ps")
                            for j in range(2):
                                nc.tensor.matmul(
                                    ps[:, j, :],
                                    kT[:, h, (kt0 + j) * 128:(kt0 + j + 1) * 128],
                                    qTg[:, h, :], start=True, stop=True,
                                )
                            et = p_et.tile([128, 2, 512], f16, tag="et")
                            nc.scalar.activation(
                                et[:], ps[:],
                                mybir.ActivationFunctionType.Exp, scale=SCALE)
                            if d >= 0:
                                meng = nc.vector if d == 0 else nc.gpsimd
                                meng.tensor_mul(et[:], et[:],
                                                masks[:, d:d + 2, :])
                            za, zeng = ((za_v, nc.vector) if step % 2 == 0
                                        else (za_p, nc.gpsimd))
                            if step < 2:
                                zinit[step % 2] = et
                            elif step < 4 and nktp > 2:
                                zeng.tensor_add(za[:], zinit.pop(step % 2)[:],
                                                et[:])
                            else:
                                zeng.tensor_add(za[:], za[:], et[:])
                            ets[step] = et
                        # deferred Z chain of previous head, issued deep into
                        # this head's stream so its za drain stays off the PE
                        # critical path
                        if step == zs1 and zchain is not None:
                            ph, pzav, pzap, ppo = zchain
                            prz = z_part1(pzav, pzap)
                            zchain = (ph, prz, ppo, True)
                        if step == zs2 and zchain is not None:
                            ph, prz, ppo, _ = zchain
                            z_part2(ph, prz, ppo, attn_g)
                            zchain = None
                        if step >= LOOKP:
                            kt0, _ = ktp_order[step - LOOKP]
                            et = ets.pop(step - LOOKP)
                            for j in range(2):
                                nc.tensor.matmul(
                                    po[:], v_sb[:, kt0 + j,
                                                h * 128:(h + 1) * 128],
                                    et[:, j, :],
                                    start=(step == LOOKP and j == 0),
                                    stop=(step == nktp + LOOKP - 1 and j == 1),
                                )
                    zav, zap = ((za_v, za_p) if nktp > 2
                                else (zinit.pop(0), zinit.pop(1)))
                    if h < HPC - 1:
                        zchain = (h, zav, zap, po)
                    else:
                        # carry last head's Z chain into the next chunk's S1
                        carryZ = (h, zav, zap, po, attn_g)
                carryO = (g, attn_g)

            # ---- epilogue: flush the last chunk's Z chain + out-projection
            ch, czav, czap, cpo, cattn = carryZ
            crz = z_part1(czav, czap)
            z_part2(ch, crz, cpo, cattn)
            cog, cattn2 = carryO
            emit_O(cog, cattn2)

    nc.compile()
    return nc


# ================= host-side prep =================

def _rope_tables(S):
    inv_freq = 1.0 / (ROPE_BASE ** (np.arange(HALF, dtype=np.float64) / HALF))
    ang = np.arange(S, dtype=np.float64)[:, None] * inv_freq[None, :]   # [S, 32]
    cosA = np.tile(np.cos(ang).T, (4, 1)).astype(np.float16)           # [128, S]
    sinA = np.tile(np.sin(ang).T, (4, 1)).astype(np.float16)
    return cosA, sinA


def _masks01():
    p = np.arange(128)[:, None]
    j = np.arange(512)[None, :]
    m = np.zeros((128, 4, 512), dtype=np.float16)
    for d in range(4):
        m[:, d, :] = (d * 128 + p <= j).astype(np.float16)
    return m


def _core_inputs(core, x, W_dq, W_dkv, W_uq, W_uk, W_uv, W_kr, W_qr, W_o, S):
    b, hg = core // 4, core % 4
    h0 = hg * HPC

    def pm(w):  # [R, C] -> [128, R//128, C] partition-major
        R, Cc = w.shape
        return np.ascontiguousarray(
            w.reshape(R // 128, 128, Cc).transpose(1, 0, 2)).astype(np.float16)

    heads = np.arange(h0, h0 + HPC)
    rope_x1 = (heads[:, None] * D_ROPE + np.arange(HALF)[None, :]).reshape(-1)
    rope_x2 = rope_x1 + HALF
    nope_cols = (heads[:, None] * D_NOPE + np.arange(D_NOPE)[None, :]).reshape(-1)
    v_cols = (heads[:, None] * D_HEAD + np.arange(D_HEAD)[None, :]).reshape(-1)

    # fold W_dq @ [W_uq | W_qr] -> direct q weights [2048, 512]
    wq_cols = np.concatenate(
        [W_uq[:, nope_cols], W_qr[:, rope_x1], W_qr[:, rope_x2]], axis=1)
    wq = (W_dq.astype(np.float64) @ wq_cols.astype(np.float64)).astype(np.float32)

    xT = np.ascontiguousarray(x[b].T)                     # [2048, S]
    TCn = S // 512
    cosA, sinA = _rope_tables(S)
    return {
        "xT": np.ascontiguousarray(
            pm(xT).reshape(128, KC, TCn, 512).transpose(2, 0, 1, 3)
        ).reshape(TCn, 128, KC * 512),
        "wq": pm(wq),
        "wdkv": np.ascontiguousarray(
            pm(W_dkv)[:, :, (core % 4) * 128:(core % 4) * 128 + 128]),
        "wkrx1": pm(W_kr[:, rope_x1]),
        "wkrx2": pm(W_kr[:, rope_x2]),
        "wuk": pm(W_uk[:, nope_cols]),
        "wuv": pm(W_uv[:, v_cols]),
        "wo": pm(W_o[h0 * D_HEAD:(h0 + HPC) * D_HEAD, :]),
        "cosA": cosA,
        "sinA": sinA,
        "masks": _masks01(),
        "onec": np.ones((128, 1), np.float16),
        "oner": np.ones((1, 128), np.float16),
    }


_NC_CACHE = {}


def _get_nc(S):
    if S not in _NC_CACHE:
        _NC_CACHE[S] = build_nc(S)
    return _NC_CACHE[S]


def make_in_maps(inputs, S):
    args = (np.asarray(inputs["x"], np.float32),
            np.asarray(inputs["W_dq"], np.float32),
            np.asarray(inputs["W_dkv"], np.float32),
            np.asarray(inputs["W_uq"], np.float32),
            np.asarray(inputs["W_uk"], np.float32),
            np.asarray(inputs["W_uv"], np.float32),
            np.asarray(inputs["W_kr"], np.float32),
            np.asarray(inputs["W_qr"], np.float32),
            np.asarray(inputs["W_o"], np.float32))
    x, W_dq, W_dkv, W_uq, W_uk, W_uv, W_kr, W_qr, W_o = args
    return [
        _core_inputs(c, x, W_dq, W_dkv, W_uq, W_uk, W_uv, W_kr, W_qr, W_o, S)
        for c in range(8)
    ]


def kernel(x, W_dkv, W_dq, W_uq, W_uk, W_uv, W_kr, W_qr, W_o, _trace=False):
    S = x.shape[1]
    nc = _get_nc(S)
    in_maps = make_in_maps(dict(x=x, W_dq=W_dq, W_dkv=W_dkv, W_uq=W_uq,
                                W_uk=W_uk, W_uv=W_uv, W_kr=W_kr, W_qr=W_qr,
                                W_o=W_o), S)
    res = bass_utils.run_bass_kernel_spmd(nc, in_maps, core_ids=list(range(8)),
                                          trace=_trace)
    out = np.zeros((B, S, H_DIM), np.float32)
    for c in range(8):
        out[c // 4] += res.results[c]["o"]
    if _trace:
        kernel.last_exec_time_ns = res.exec_time_ns
        kernel.last_results = res
    return out


# revision 36
# speedup vs baseline: 1.0318x; 1.0318x over previous
"""MLA (multi-head latent attention) Trainium2 Bass kernel, 8-core SPMD.

Sharding: 2-way data parallel over batch x 4-way tensor parallel over heads.
Core c handles batch b = c // 4 and heads [hg*4, hg*4+4) with hg = c % 4.
Each core computes the full MLA forward for its batch/heads and the partial
out-projection (row-sharded W_o); the host sums the 4 partials per batch.

v2: fully fused per-chunk pipeline. For each 512-token chunk g:
  S1: q (folded W_dq@[W_uq|W_qr], direct from x), c_kv, krx chains
  S2: k up-proj + rope, v up-proj
  A(g): causal attention for query chunk g over key chunks 0..g
  O(g): out-projection, DMA'd straight from PSUM
Engine balance: PE does all matmuls; Scalar does exp + PSUM evictions;
Pool (gpsimd) does causal 0/1-mask muls + softmax-Z accumulation; Vector
does rope + reciprocal + final normalize mul. Scores are issued 2 tiles
ahead of the AV matmuls and each head's Z-normalize chain is deferred into
the next head's score stream so the PE never waits on exp.

Layouts on device (partition dim first):
  xT      [128, 16, S]   x[b].T, feature-on-partition
  qTg     [128, 4, 512]  per head: rows 0:64 content, 64:96/96:128 rope halves
  kT      [128, 4, S]    same row layout
  v       [128, 16, 512] [token%128, token//128, head*128+d]
  scoresT [128k, 512q]   PSUM; exp'd on Scalar -> et (fp16); Z via pool adds
                         + ones-matmul; normalize at attn eviction.
All matmul operands are fp16 (1 cyc/row at full PE clock).
"""
import sys

sys.path.insert(0, "/opt/trn_rl_repo")

import numpy as np

import concourse.bacc as bacc
import concourse.mybir as mybir
import concourse.tile as tile
from concourse import bass_utils

H_DIM = 2048
N_HEADS = 16
D_HEAD = 128
D_ROPE = 64
D_NOPE = 64
HALF = D_ROPE // 2          # 32
C_DIM = 512
ROPE_BASE = 10000.0
HPC = 4                     # heads per core
B = 2
S_FULL = 2048
KC = H_DIM // 128           # 16
CC = C_DIM // 128           # 4
SCALE = 1.0 / float(np.sqrt(D_HEAD))

f16 = mybir.dt.float16
f32 = mybir.dt.float32


def build_nc(S=S_FULL):
    TC = S // 512            # token chunks / query groups

    nc = bacc.Bacc("TRN2", target_bir_lowering=False, debug=False)

    d_xT = nc.dram_tensor("xT", [TC, 128, KC * 512], f16, kind="ExternalInput")
    d_wq = nc.dram_tensor("wq", [128, KC, 512], f16, kind="ExternalInput")
    d_wdkv = nc.dram_tensor("wdkv", [128, KC, C_DIM], f16, kind="ExternalInput")
    d_wkrx1 = nc.dram_tensor("wkrx1", [128, KC, HPC * HALF], f16, kind="ExternalInput")
    d_wkrx2 = nc.dram_tensor("wkrx2", [128, KC, HPC * HALF], f16, kind="ExternalInput")
    d_wuk = nc.dram_tensor("wuk", [128, CC, HPC * D_NOPE], f16, kind="ExternalInput")
    d_wuv = nc.dram_tensor("wuv", [128, CC, HPC * D_HEAD], f16, kind="ExternalInput")
    d_wo = nc.dram_tensor("wo", [128, HPC, H_DIM], f16, kind="ExternalInput")
    d_cos = nc.dram_tensor("cosA", [128, S], f16, kind="ExternalInput")
    d_sin = nc.dram_tensor("sinA", [128, S], f16, kind="ExternalInput")
    d_mask = nc.dram_tensor("masks", [128, 4, 512], f16, kind="ExternalInput")
    d_onec = nc.dram_tensor("onec", [128, 1], f16, kind="ExternalInput")
    d_oner = nc.dram_tensor("oner", [1, 128], f16, kind="ExternalInput")
    d_o = nc.dram_tensor("o", [S, H_DIM], f32, kind="ExternalOutput")

    import contextlib
    with tile.TileContext(nc) as tc:
        with contextlib.ExitStack() as stack:
            def pool(name, **kw):
                return stack.enter_context(tc.tile_pool(name=name, **kw))

            p_const = pool("const", bufs=1)
            p_w = pool("w", bufs=1, side="right")
            p_x = pool("xp", bufs=2, side="right")
            p_kT = pool("kT", bufs=1)
            p_v = pool("vp", bufs=1)
            p_qT = pool("qT", bufs=2)
            p_ckv = pool("ckv", bufs=2, side="right")
            p_krx = pool("krx", bufs=1, side="right")
            p_tmp = pool("tmp", bufs=1, side="right")
            p_et = pool("et", bufs=4)
            p_za = pool("za", bufs=2)
            p_rz = pool("rz", bufs=1)
            p_rb = pool("rb", bufs=1)
            p_att = pool("att", bufs=2)
            p_ot = pool("ot", bufs=2)
            p_psA = pool("psA", bufs=2, space="PSUM")
            p_po = pool("ps_po", bufs=3, space="PSUM")
            p_pz = pool("ps_z", bufs=1, space="PSUM")

            # ---- constants + weights, ordered by first use. x chunks go on
            # the scalar engine's DMA queue so they overlap the weight DMAs
            # (sync queue).
            # Queue plan: scalar queue carries ONLY x chunks (chunk 0 first,
            # nothing ahead of it); sync queue carries the weights in first-use
            # order; the gpsimd SWDGE queue carries the small / late tensors.
            wq = p_w.tile([128, KC, 512], f16, tag="wq")
            nc.sync.dma_start(wq[:, 0:KC // 2, :], d_wq.ap()[:, 0:KC // 2, :])
            nc.gpsimd.dma_start(wq[:, KC // 2:KC, :],
                                d_wq.ap()[:, KC // 2:KC, :])
            wdkv = p_w.tile([128, KC, C_DIM], f16, tag="wdkv")
            nc.sync.dma_start(wdkv[:], d_wdkv.ap())
            wkrx1 = p_w.tile([128, KC, HPC * HALF], f16, tag="wkrx1")
            nc.sync.dma_start(wkrx1[:], d_wkrx1.ap())
            wkrx2 = p_w.tile([128, KC, HPC * HALF], f16, tag="wkrx2")
            nc.sync.dma_start(wkrx2[:], d_wkrx2.ap())
            wo = p_w.tile([128, HPC, H_DIM], f16, tag="wo")
            nc.sync.dma_start(wo[:], d_wo.ap())
            cosA = p_const.tile([128, S], f16, tag="cosA")
            nc.gpsimd.dma_start(cosA[:], d_cos.ap())
            sinA = p_const.tile([128, S], f16, tag="sinA")
            nc.gpsimd.dma_start(sinA[:], d_sin.ap())
            wuk = p_w.tile([128, CC, HPC * D_NOPE], f16, tag="wuk")
            nc.gpsimd.dma_start(wuk[:], d_wuk.ap())
            wuv = p_w.tile([128, CC, HPC * D_HEAD], f16, tag="wuv")
            nc.gpsimd.dma_start(wuv[:], d_wuv.ap())
            masks = p_const.tile([128, 4, 512], f16, tag="masks")
            nc.gpsimd.dma_start(masks[:], d_mask.ap())
            onech = p_const.tile([128, 1], f16, tag="onec")
            nc.gpsimd.dma_start(onech[:], d_onec.ap())

            # persistent K/V for all chunks
            kT = p_kT.tile([128, HPC, S], f16, tag="kT")
            v_sb = p_v.tile([128, S // 128, 512], f16, tag="v")

            def rope_psum(x1ap, x2ap, dst, gs, dc):
                # x1/x2: [128 = 4h*32, 512] (psum or sbuf); writes rope rows
                t1 = p_tmp.tile([128, 512], f16, tag="t1")
                t2 = p_tmp.tile([128, 512], f16, tag="t2")
                o1 = p_tmp.tile([128, 512], f16, tag="o1")
                o2 = p_tmp.tile([128, 512], f16, tag="o2")
                nc.vector.tensor_mul(t1[:], x1ap, cosA[:, gs])
                nc.vector.tensor_mul(t2[:], x2ap, sinA[:, gs])
                nc.vector.tensor_sub(o1[:], t1[:], t2[:])
                nc.vector.tensor_mul(t1[:], x1ap, sinA[:, gs])
                nc.vector.tensor_mul(t2[:], x2ap, cosA[:, gs])
                nc.vector.tensor_add(o2[:], t1[:], t2[:])
                for h in range(HPC):
                    hs = slice(h * HALF, (h + 1) * HALF)
                    nc.scalar.copy(dst[64:96, h, dc], o1[hs, :])
                    nc.scalar.copy(dst[96:128, h, dc], o2[hs, :])

            def z_part1(za_v, za_p):
                # zr[q]: (za_v + za_p) then summed over the pair dim
                zm = p_rb.tile([128, 2, 512], f16, tag="zm")
                nc.vector.tensor_add(zm[:], za_v[:], za_p[:])
                zr = p_rb.tile([128, 512], f16, tag="zr")
                nc.vector.tensor_add(zr[:], zm[:, 0, :], zm[:, 1, :])
                pz = p_pz.tile([1, 512], f32, tag="pz")
                nc.tensor.matmul(pz[:], onech[:], zr[:], start=True,
                                 stop=True)
                r0 = p_rz.tile([1, 512], f32, tag="r0")
                nc.vector.reciprocal(r0[:], pz[:])
                rzc = p_rz.tile([1, 512], f16, tag="rzc")
                nc.vector.tensor_copy(rzc[:], r0[:])
                return rzc

            def z_part2(h, rzc, po, attn_t):
                rb = p_rb.tile([128, 512], f16, tag="rb")
                nc.gpsimd.partition_broadcast(rb[:], rzc[:])
                nc.vector.tensor_mul(attn_t[:, h, :], po[:], rb[:])

            def emit_O(og, attn_t):
                # out-projection for chunk og; nck pairs share one 2-bank
                # PSUM tile, evicted with a single copy + single DMA
                for t4 in range(4):
                    tt = og * 4 + t4
                    for np2 in range(2):
                        pso = p_psA.tile([128, 2, 512], f32, tag="ps")
                        for j in range(2):
                            nck = 2 * np2 + j
                            for h in range(HPC):
                                nc.tensor.matmul(
                                    pso[:, j, :],
                                    attn_t[:, h, t4 * 128:(t4 + 1) * 128],
                                    wo[:, h, nck * 512:(nck + 1) * 512],
                                    start=(h == 0), stop=(h == HPC - 1),
                                )
                        ot = p_ot.tile([128, 2, 512], f32, tag="ot")
                        if (t4 + np2) % 2 == 0:
                            nc.vector.tensor_copy(ot[:], pso[:])
                        else:
                            nc.scalar.copy(ot[:], pso[:])
                        nc.sync.dma_start(
                            d_o.ap()[tt * 128:(tt + 1) * 128,
                                     np2 * 1024:(np2 + 1) * 1024],
                            ot[:])

            carryZ = None   # last head of previous chunk: (h, za_v, za_p, po, attn)
            carryO = None   # previous chunk's out-projection: (g, attn)

            for g in range(TC):
                gs = slice(g * 512, (g + 1) * 512)
                xt = p_x.tile([128, KC, 512], f16, tag="xt")
                if g == 0:
                    nc.scalar.dma_start(xt[:, 0:KC // 2, :],
                                        d_xT.ap()[0][:, 0:(KC // 2) * 512])
                    nc.scalar.dma_start(xt[:, KC // 2:KC, :],
                                        d_xT.ap()[0][:, (KC // 2) * 512:])
                else:
                    nc.scalar.dma_start(xt[:], d_xT.ap()[g])

                # ---- S1a: q direct from x (folded weights) ----
                # The previous chunk's last-head Z chain is issued between the
                # first chains so the PE never waits on it.
                qTg = p_qT.tile([128, HPC, 512], f16, tag="qTg")
                qrope = None
                for mp in range(2):
                    ps = p_psA.tile([128, 2, 512], f32, tag="ps")
                    for j in range(2):
                        m = 2 * mp + j
                        for k in range(KC):
                            nc.tensor.matmul(
                                ps[:, j, :], wq[:, k, m * 128:(m + 1) * 128],
                                xt[:, k, :],
                                start=(k == 0), stop=(k == KC - 1),
                            )
                    if mp == 0:
                        for hh in range(4):
                            nc.scalar.copy(
                                qTg[0:64, hh, :],
                                ps[64 * (hh % 2):64 * (hh % 2) + 64, hh // 2, :])
                        if carryZ is not None:
                            ch, czav, czap, cpo, cattn = carryZ
                            crz = z_part1(czav, czap)
                            carryZ = (ch, crz, cpo, cattn)
                    else:
                        qrope = ps
                        if carryZ is not None:
                            ch, crz, cpo, cattn = carryZ
                            z_part2(ch, crz, cpo, cattn)
                            carryZ = None
                rope_psum(qrope[:, 0, :], qrope[:, 1, :], qTg, gs, slice(0, 512))

                # ---- S1b: c_kv chains ----
                ckvg = p_ckv.tile([128, CC, 512], f16, tag="ckvg")
                for mp in range(2):
                    ps = p_psA.tile([128, 2, 512], f32, tag="ps")
                    for j in range(2):
                        m = 2 * mp + j
                        for k in range(KC):
                            nc.tensor.matmul(
                                ps[:, j, :], wdkv[:, k, m * 128:(m + 1) * 128],
                                xt[:, k, :],
                                start=(k == 0), stop=(k == KC - 1),
                            )
                    nc.scalar.copy(ckvg[:, 2 * mp:2 * mp + 2, :], ps[:])

                # ---- S1c: krx chains ----
                kx = p_krx.tile([128, 2, 512], f16, tag="kx")
                ps = p_psA.tile([128, 2, 512], f32, tag="ps")
                for j, w_sb in enumerate((wkrx1, wkrx2)):
                    for k in range(KC):
                        nc.tensor.matmul(
                            ps[:, j, :], w_sb[:, k, :], xt[:, k, :],
                            start=(k == 0), stop=(k == KC - 1),
                        )
                nc.scalar.copy(kx[:], ps[:])

                # ---- S2: k up-proj (content) + rope; v up-proj ----
                ps = p_psA.tile([128, 2, 512], f32, tag="ps")
                for m2 in range(2):
                    for k in range(CC):
                        nc.tensor.matmul(
                            ps[:, m2, :], wuk[:, k, m2 * 128:(m2 + 1) * 128],
                            ckvg[:, k, :], start=(k == 0), stop=(k == CC - 1),
                        )
                for hh in range(4):
                    nc.scalar.copy(
                        kT[0:64, hh, gs],
                        ps[64 * (hh % 2):64 * (hh % 2) + 64, hh // 2, :])
                rope_psum(kx[:, 0, :], kx[:, 1, :], kT, gs, gs)

                for tp in range(2):
                    ps = p_psA.tile([128, 2, 512], f32, tag="ps")
                    for j in range(2):
                        tt = 2 * tp + j
                        for k in range(CC):
                            nc.tensor.matmul(
                                ps[:, j, :], ckvg[:, k, tt * 128:(tt + 1) * 128],
                                wuv[:, k, :], start=(k == 0), stop=(k == CC - 1),
                            )
                    nc.scalar.copy(v_sb[:, g * 4 + 2 * tp:g * 4 + 2 * tp + 2, :],
                                   ps[:])

                # ---- O(g-1): previous chunk's out-projection ----
                if carryO is not None:
                    cog, cattn = carryO
                    emit_O(cog, cattn)
                    carryO = None

                # ---- A(g): attention for query chunk g ----
                # kt order: diagonal (masked) tiles first so their longer
                # exp->mask chain hides under the unmasked tiles' stream.
                # Z accumulates in two tiles (vector even / pool odd) to halve
                # the serial add-chain; the normalize chain of head h is
                # issued inside head h+1's score stream.
                attn_g = p_att.tile([128, HPC, 512], f16, tag="attn")
                nkt = 4 * (g + 1)
                nktp = nkt // 2
                LOOKP = 2
                # kt pairs, diagonal (masked) pairs first
                ktp_order = ([(4 * g, 0), (4 * g + 2, 2)]
                             + [(2 * i, -1) for i in range(2 * g)])
                zchain = None

                zs1 = min(2, nktp)
                zs2 = min(4, nktp + LOOKP - 1)
                for h in range(HPC):
                    po = p_po.tile([128, 512], f32, tag="po")
                    za_v = za_p = None
                    if nktp > 2:
                        za_v = p_za.tile([128, 2, 512], f32, tag="za_v")
                        za_p = p_za.tile([128, 2, 512], f32, tag="za_p")
                    ets = {}
                    zinit = {}
                    for step in range(nktp + LOOKP):
                        if step < nktp:
                            kt0, d = ktp_order[step]
                            ps = p_psA.tile([128, 2, 512], f32, tag="ps")
                            for j in range(2):
                                nc.tensor.matmul(
                                    ps[:, j, :],
                                    kT[:, h, (kt0 + j) * 128:(kt0 + j + 1) * 128],
                                    qTg[:, h, :], start=True, stop=True,
                                )
                            et = p_et.tile([128, 2, 512], f16, tag="et")
                            nc.scalar.activation(
                                et[:], ps[:],
                                mybir.ActivationFunctionType.Exp, scale=SCALE)
                            if d >= 0:
                                meng = nc.vector if d == 0 else nc.gpsimd
                                meng.tensor_mul(et[:], et[:],
                                                masks[:, d:d + 2, :])
                            za, zeng = ((za_v, nc.vector) if step % 2 == 0
                                        else (za_p, nc.gpsimd))
                            if step < 2:
                                zinit[step % 2] = et
                            elif step < 4 and nktp > 2:
                                zeng.tensor_add(za[:], zinit.pop(step % 2)[:],
                                                et[:])
                            else:
                                zeng.tensor_add(za[:], za[:], et[:])
                            ets[step] = et
                        # deferred Z chain of previous head, issued deep into
                        # this head's stream so its za drain stays off the PE
                        # critical path
                        if step == zs1 and zchain is not None:
                            ph, pzav, pzap, ppo = zchain
                            prz = z_part1(pzav, pzap)
                            zchain = (ph, prz, ppo, True)
                        if step == zs2 and zchain is not None:
                            ph, prz, ppo, _ = zchain
                            z_part2(ph, prz, ppo, attn_g)
                            zchain = None
                        if step >= LOOKP:
                            kt0, _ = ktp_order[step - LOOKP]
                            et = ets.pop(step - LOOKP)
                            for j in range(2):
                                nc.tensor.matmul(
                                    po[:], v_sb[:, kt0 + j,
                                                h * 128:(h + 1) * 128],
                                    et[:, j, :],
                                    start=(step == LOOKP and j == 0),
                                    stop=(step == nktp + LOOKP - 1 and j == 1),
                                )
                    zav, zap = ((za_v, za_p) if nktp > 2
                                else (zinit.pop(0), zinit.pop(1)))
                    if h < HPC - 1:
                        zchain = (h, zav, zap, po)
                    else:
                        # carry last head's Z chain into the next chunk's S1
                        carryZ = (h, zav, zap, po, attn_g)
                carryO = (g, attn_g)

            # ---- epilogue: flush the last chunk's Z chain + out-projection
            ch, czav, czap, cpo, cattn = carryZ
            crz = z_part1(czav, czap)
            z_part2(ch, crz, cpo, cattn)
            cog, cattn2 = carryO
            emit_O(cog, cattn2)

    nc.compile()
    return nc


# ================= host-side prep =================

def _rope_tables(S):
    inv_freq = 1.0 / (ROPE_BASE ** (np.arange(HALF, dtype=np.float64) / HALF))
    ang = np.arange(S, dtype=np.float64)[:, None] * inv_freq[None, :]   # [S, 32]
    cosA = np.tile(np.cos(ang).T, (4, 1)).astype(np.float16)           # [128, S]
    sinA = np.tile(np.sin(ang).T, (4, 1)).astype(np.float16)
    return cosA, sinA


def _masks01():
    p = np.arange(128)[:, None]
    j = np.arange(512)[None, :]
    m = np.zeros((128, 4, 512), dtype=np.float16)
    for d in range(4):
        m[:, d, :] = (d * 128 + p <= j).astype(np.float16)
    return m


def _core_inputs(core, x, W_dq, W_dkv, W_uq, W_uk, W_uv, W_kr, W_qr, W_o, S):
    b, hg = core // 4, core % 4
    h0 = hg * HPC

    def pm(w):  # [R, C] -> [128, R//128, C] partition-major
        R, Cc = w.shape
        return np.ascontiguousarray(
            w.reshape(R // 128, 128, Cc).transpose(1, 0, 2)).astype(np.float16)

    heads = np.arange(h0, h0 + HPC)
    rope_x1 = (heads[:, None] * D_ROPE + np.arange(HALF)[None, :]).reshape(-1)
    rope_x2 = rope_x1 + HALF
    nope_cols = (heads[:, None] * D_NOPE + np.arange(D_NOPE)[None, :]).reshape(-1)
    v_cols = (heads[:, None] * D_HEAD + np.arange(D_HEAD)[None, :]).reshape(-1)

    # fold W_dq @ [W_uq | W_qr] -> direct q weights [2048, 512]
    wq_cols = np.concatenate(
        [W_uq[:, nope_cols], W_qr[:, rope_x1], W_qr[:, rope_x2]], axis=1)
    wq = (W_dq.astype(np.float64) @ wq_cols.astype(np.float64)).astype(np.float32)

    xT = np.ascontiguousarray(x[b].T)                     # [2048, S]
    TCn = S // 512
    cosA, sinA = _rope_tables(S)
    return {
        "xT": np.ascontiguousarray(
            pm(xT).reshape(128, KC, TCn, 512).transpose(2, 0, 1, 3)
        ).reshape(TCn, 128, KC * 512),
        "wq": pm(wq),
        "wdkv": pm(W_dkv),
        "wkrx1": pm(W_kr[:, rope_x1]),
        "wkrx2": pm(W_kr[:, rope_x2]),
        "wuk": pm(W_uk[:, nope_cols]),
        "wuv": pm(W_uv[:, v_cols]),
        "wo": pm(W_o[h0 * D_HEAD:(h0 + HPC) * D_HEAD, :]),
        "cosA": cosA,
        "sinA": sinA,
        "masks": _masks01(),
        "onec": np.ones((128, 1), np.float16),
        "oner": np.ones((1, 128), np.float16),
    }


_NC_CACHE = {}


def _get_nc(S):
    if S not in _NC_CACHE:
        _NC_CACHE[S] = build_nc(S)
    return _NC_CACHE[S]


def make_in_maps(inputs, S):
    args = (np.asarray(inputs["x"], np.float32),
            np.asarray(inputs["W_dq"], np.float32),
            np.asarray(inputs["W_dkv"], np.float32),
            np.asarray(inputs["W_uq"], np.float32),
            np.asarray(inputs["W_uk"], np.float32),
            np.asarray(inputs["W_uv"], np.float32),
            np.asarray(inputs["W_kr"], np.float32),
            np.asarray(inputs["W_qr"], np.float32),
            np.asarray(inputs["W_o"], np.float32))
    x, W_dq, W_dkv, W_uq, W_uk, W_uv, W_kr, W_qr, W_o = args
    return [
        _core_inputs(c, x, W_dq, W_dkv, W_uq, W_uk, W_uv, W_kr, W_qr, W_o, S)
        for c in range(8)
    ]


def kernel(x, W_dkv, W_dq, W_uq, W_uk, W_uv, W_kr, W_qr, W_o, _trace=False):
    S = x.shape[1]
    nc = _get_nc(S)
    in_maps = make_in_maps(dict(x=x, W_dq=W_dq, W_dkv=W_dkv, W_uq=W_uq,
                                W_uk=W_uk, W_uv=W_uv, W_kr=W_kr, W_qr=W_qr,
                                W_o=W_o), S)
    res = bass_utils.run_bass_kernel_spmd(nc, in_maps, core_ids=list(range(8)),
                                          trace=_trace)
    out = np.zeros((B, S, H_DIM), np.float32)
    for c in range(8):
        out[c // 4] += res.results[c]["o"]
    if _trace:
        kernel.last_exec_time_ns = res.exec_time_ns
        kernel.last_results = res
    return out


# revision 38
# speedup vs baseline: 1.1814x; 1.1450x over previous
"""MLA (multi-head latent attention) Trainium2 Bass kernel, 8-core SPMD.

Sharding: 2-way data parallel over batch x 4-way tensor parallel over heads.
Core c handles batch b = c // 4 and heads [hg*4, hg*4+4) with hg = c % 4.
Each core computes the full MLA forward for its batch/heads and the partial
out-projection (row-sharded W_o); the host sums the 4 partials per batch.

v2: fully fused per-chunk pipeline. For each 512-token chunk g:
  S1: q (folded W_dq@[W_uq|W_qr], direct from x), c_kv, krx chains
  S2: k up-proj + rope, v up-proj
  A(g): causal attention for query chunk g over key chunks 0..g
  O(g): out-projection, DMA'd straight from PSUM
Engine balance: PE does all matmuls; Scalar does exp + PSUM evictions;
Pool (gpsimd) does causal 0/1-mask muls + softmax-Z accumulation; Vector
does rope + reciprocal + final normalize mul. Scores are issued 2 tiles
ahead of the AV matmuls and each head's Z-normalize chain is deferred into
the next head's score stream so the PE never waits on exp.

Layouts on device (partition dim first):
  xT      [128, 16, S]   x[b].T, feature-on-partition
  qTg     [128, 4, 512]  per head: rows 0:64 content, 64:96/96:128 rope halves
  kT      [128, 4, S]    same row layout
  v       [128, 16, 512] [token%128, token//128, head*128+d]
  scoresT [128k, 512q]   PSUM; exp'd on Scalar -> et (fp16); Z via pool adds
                         + ones-matmul; normalize at attn eviction.
All matmul operands are fp16 (1 cyc/row at full PE clock).
"""
import sys

sys.path.insert(0, "/opt/trn_rl_repo")

import numpy as np

import concourse.bacc as bacc
import concourse.mybir as mybir
import concourse.tile as tile
from concourse import bass_utils

H_DIM = 2048
N_HEADS = 16
D_HEAD = 128
D_ROPE = 64
D_NOPE = 64
HALF = D_ROPE // 2          # 32
C_DIM = 512
ROPE_BASE = 10000.0
HPC = 4                     # heads per core
B = 2
S_FULL = 2048
KC = H_DIM // 128           # 16
CC = C_DIM // 128           # 4
SCALE = 1.0 / float(np.sqrt(D_HEAD))

f16 = mybir.dt.float16
f32 = mybir.dt.float32


def build_nc(S=S_FULL):
    TC = S // 512            # token chunks / query groups

    nc = bacc.Bacc("TRN2", target_bir_lowering=False, debug=False)

    d_xT = nc.dram_tensor("xT", [TC, 128, KC * 512], f16, kind="ExternalInput")
    d_wq = nc.dram_tensor("wq", [128, KC, 512], f16, kind="ExternalInput")
    d_wdkv = nc.dram_tensor("wdkv", [128, KC, C_DIM], f16, kind="ExternalInput")
    d_wkrx1 = nc.dram_tensor("wkrx1", [128, KC, HPC * HALF], f16, kind="ExternalInput")
    d_wkrx2 = nc.dram_tensor("wkrx2", [128, KC, HPC * HALF], f16, kind="ExternalInput")
    d_wuk = nc.dram_tensor("wuk", [128, CC, HPC * D_NOPE], f16, kind="ExternalInput")
    d_wuv = nc.dram_tensor("wuv", [128, CC, HPC * D_HEAD], f16, kind="ExternalInput")
    d_wo = nc.dram_tensor("wo", [128, HPC, H_DIM], f16, kind="ExternalInput")
    d_cos = nc.dram_tensor("cosA", [128, S], f16, kind="ExternalInput")
    d_sin = nc.dram_tensor("sinA", [128, S], f16, kind="ExternalInput")
    d_mask = nc.dram_tensor("masks", [128, 4, 512], f16, kind="ExternalInput")
    d_onec = nc.dram_tensor("onec", [128, 1], f16, kind="ExternalInput")
    d_oner = nc.dram_tensor("oner", [1, 128], f16, kind="ExternalInput")
    d_o = nc.dram_tensor("o", [S, H_DIM], f32, kind="ExternalOutput")

    import contextlib
    with tile.TileContext(nc) as tc:
        with contextlib.ExitStack() as stack:
            def pool(name, **kw):
                return stack.enter_context(tc.tile_pool(name=name, **kw))

            p_const = pool("const", bufs=1)
            p_w = pool("w", bufs=1, side="right")
            p_x = pool("xp", bufs=2, side="right")
            p_kT = pool("kT", bufs=1)
            p_v = pool("vp", bufs=1)
            p_qT = pool("qT", bufs=2)
            p_ckv = pool("ckv", bufs=2, side="right")
            p_krx = pool("krx", bufs=1, side="right")
            p_tmp = pool("tmp", bufs=1, side="right")
            p_et = pool("et", bufs=4)
            p_za = pool("za", bufs=2)
            p_rz = pool("rz", bufs=1)
            p_rb = pool("rb", bufs=1)
            p_att = pool("att", bufs=2)
            p_ot = pool("ot", bufs=2)
            p_psA = pool("psA", bufs=2, space="PSUM")
            p_po = pool("ps_po", bufs=2, space="PSUM")
            p_pz = pool("ps_z", bufs=1, space="PSUM")
            p_pb = pool("ps_b", bufs=1, space="PSUM")

            # ---- constants + weights, ordered by first use. x chunks go on
            # the scalar engine's DMA queue so they overlap the weight DMAs
            # (sync queue).
            # Queue plan: scalar queue carries ONLY x chunks (chunk 0 first,
            # nothing ahead of it); sync queue carries the weights in first-use
            # order; the gpsimd SWDGE queue carries the small / late tensors.
            wq = p_w.tile([128, KC, 512], f16, tag="wq")
            nc.sync.dma_start(wq[:], d_wq.ap())
            wdkv = p_w.tile([128, KC, C_DIM], f16, tag="wdkv")
            nc.sync.dma_start(wdkv[:], d_wdkv.ap())
            wkrx1 = p_w.tile([128, KC, HPC * HALF], f16, tag="wkrx1")
            nc.sync.dma_start(wkrx1[:], d_wkrx1.ap())
            wkrx2 = p_w.tile([128, KC, HPC * HALF], f16, tag="wkrx2")
            nc.sync.dma_start(wkrx2[:], d_wkrx2.ap())
            wo = p_w.tile([128, HPC, H_DIM], f16, tag="wo")
            nc.sync.dma_start(wo[:], d_wo.ap())
            cosA = p_const.tile([128, S], f16, tag="cosA")
            nc.gpsimd.dma_start(cosA[:], d_cos.ap())
            sinA = p_const.tile([128, S], f16, tag="sinA")
            nc.gpsimd.dma_start(sinA[:], d_sin.ap())
            wuk = p_w.tile([128, CC, HPC * D_NOPE], f16, tag="wuk")
            nc.gpsimd.dma_start(wuk[:], d_wuk.ap())
            wuv = p_w.tile([128, CC, HPC * D_HEAD], f16, tag="wuv")
            nc.gpsimd.dma_start(wuv[:], d_wuv.ap())
            masks = p_const.tile([128, 4, 512], f16, tag="masks")
            nc.gpsimd.dma_start(masks[:], d_mask.ap())
            onech = p_const.tile([128, 1], f16, tag="onec")
            nc.gpsimd.dma_start(onech[:], d_onec.ap())
            oner = p_const.tile([1, 128], f16, tag="oner")
            nc.gpsimd.dma_start(oner[:], d_oner.ap())

            # persistent K/V for all chunks
            kT = p_kT.tile([128, HPC, S], f16, tag="kT")
            v_sb = p_v.tile([128, S // 128, 512], f16, tag="v")

            def rope_psum(x1ap, x2ap, dst, gs, dc):
                # x1/x2: [128 = 4h*32, 512] (psum or sbuf); writes rope rows
                t1 = p_tmp.tile([128, 512], f16, tag="t1")
                t2 = p_tmp.tile([128, 512], f16, tag="t2")
                o1 = p_tmp.tile([128, 512], f16, tag="o1")
                o2 = p_tmp.tile([128, 512], f16, tag="o2")
                nc.vector.tensor_mul(t1[:], x1ap, cosA[:, gs])
                nc.vector.tensor_mul(t2[:], x2ap, sinA[:, gs])
                nc.vector.tensor_sub(o1[:], t1[:], t2[:])
                nc.vector.tensor_mul(t1[:], x1ap, sinA[:, gs])
                nc.vector.tensor_mul(t2[:], x2ap, cosA[:, gs])
                nc.vector.tensor_add(o2[:], t1[:], t2[:])
                for h in range(HPC):
                    hs = slice(h * HALF, (h + 1) * HALF)
                    nc.scalar.copy(dst[64:96, h, dc], o1[hs, :])
                    nc.scalar.copy(dst[96:128, h, dc], o2[hs, :])

            def z_part1(za_v, za_p):
                # zr[q]: (za_v + za_p) then summed over the pair dim
                zm = p_rb.tile([128, 2, 512], f16, tag="zm")
                nc.vector.tensor_add(zm[:], za_v[:], za_p[:])
                zr = p_rb.tile([128, 512], f16, tag="zr")
                nc.vector.tensor_add(zr[:], zm[:, 0, :], zm[:, 1, :])
                pz = p_pz.tile([1, 512], f32, tag="pz")
                nc.tensor.matmul(pz[:], onech[:], zr[:], start=True,
                                 stop=True)
                r0 = p_rz.tile([1, 512], f32, tag="r0")
                nc.vector.reciprocal(r0[:], pz[:])
                rzc = p_rz.tile([1, 512], f16, tag="rzc")
                nc.vector.tensor_copy(rzc[:], r0[:])
                return rzc

            def z_part2(h, rzc, po, attn_t):
                pb = p_pb.tile([128, 512], f32, tag="pb")
                nc.tensor.matmul(pb[:], oner[:], rzc[:], start=True, stop=True)
                rb = p_rb.tile([128, 512], f16, tag="rb")
                nc.scalar.copy(rb[:], pb[:])
                nc.vector.tensor_mul(attn_t[:, h, :], po[:], rb[:])

            def emit_O(og, attn_t):
                # out-projection for chunk og; nck pairs share one 2-bank
                # PSUM tile, evicted with a single copy + single DMA
                for t4 in range(4):
                    tt = og * 4 + t4
                    for np2 in range(2):
                        pso = p_psA.tile([128, 2, 512], f32, tag="ps")
                        for j in range(2):
                            nck = 2 * np2 + j
                            for h in range(HPC):
                                nc.tensor.matmul(
                                    pso[:, j, :],
                                    attn_t[:, h, t4 * 128:(t4 + 1) * 128],
                                    wo[:, h, nck * 512:(nck + 1) * 512],
                                    start=(h == 0), stop=(h == HPC - 1),
                                )
                        ot = p_ot.tile([128, 2, 512], f32, tag="ot")
                        if (t4 + np2) % 2 == 0:
                            nc.vector.tensor_copy(ot[:], pso[:])
                        else:
                            nc.scalar.copy(ot[:], pso[:])
                        nc.sync.dma_start(
                            d_o.ap()[tt * 128:(tt + 1) * 128,
                                     np2 * 1024:(np2 + 1) * 1024],
                            ot[:])

            carryZ = None   # last head of previous chunk: (h, za_v, za_p, po, attn)
            carryO = None   # previous chunk's out-projection: (g, attn)

            for g in range(TC):
                gs = slice(g * 512, (g + 1) * 512)
                xt = p_x.tile([128, KC, 512], f16, tag="xt")
                nc.scalar.dma_start(xt[:], d_xT.ap()[g])

                # ---- S1a: q direct from x (folded weights) ----
                # The previous chunk's last-head Z chain is issued between the
                # first chains so the PE never waits on it.
                qTg = p_qT.tile([128, HPC, 512], f16, tag="qTg")
                qrope = None
                for mp in range(2):
                    ps = p_psA.tile([128, 2, 512], f32, tag="ps")
                    for j in range(2):
                        m = 2 * mp + j
                        for k in range(KC):
                            nc.tensor.matmul(
                                ps[:, j, :], wq[:, k, m * 128:(m + 1) * 128],
                                xt[:, k, :],
                                start=(k == 0), stop=(k == KC - 1),
                            )
                    if mp == 0:
                        for hh in range(4):
                            nc.scalar.copy(
                                qTg[0:64, hh, :],
                                ps[64 * (hh % 2):64 * (hh % 2) + 64, hh // 2, :])
                        if carryZ is not None:
                            ch, czav, czap, cpo, cattn = carryZ
                            crz = z_part1(czav, czap)
                            carryZ = (ch, crz, cpo, cattn)
                    else:
                        qrope = ps
                        if carryZ is not None:
                            ch, crz, cpo, cattn = carryZ
                            z_part2(ch, crz, cpo, cattn)
                            carryZ = None
                rope_psum(qrope[:, 0, :], qrope[:, 1, :], qTg, gs, slice(0, 512))

                # ---- S1b: c_kv chains ----
                ckvg = p_ckv.tile([128, CC, 512], f16, tag="ckvg")
                for mp in range(2):
                    ps = p_psA.tile([128, 2, 512], f32, tag="ps")
                    for j in range(2):
                        m = 2 * mp + j
                        for k in range(KC):
                            nc.tensor.matmul(
                                ps[:, j, :], wdkv[:, k, m * 128:(m + 1) * 128],
                                xt[:, k, :],
                                start=(k == 0), stop=(k == KC - 1),
                            )
                    nc.scalar.copy(ckvg[:, 2 * mp:2 * mp + 2, :], ps[:])

                # ---- S1c: krx chains ----
                kx = p_krx.tile([128, 2, 512], f16, tag="kx")
                ps = p_psA.tile([128, 2, 512], f32, tag="ps")
                for j, w_sb in enumerate((wkrx1, wkrx2)):
                    for k in range(KC):
                        nc.tensor.matmul(
                            ps[:, j, :], w_sb[:, k, :], xt[:, k, :],
                            start=(k == 0), stop=(k == KC - 1),
                        )
                nc.scalar.copy(kx[:], ps[:])

                # ---- S2: k up-proj (content) + rope; v up-proj ----
                ps = p_psA.tile([128, 2, 512], f32, tag="ps")
                for m2 in range(2):
                    for k in range(CC):
                        nc.tensor.matmul(
                            ps[:, m2, :], wuk[:, k, m2 * 128:(m2 + 1) * 128],
                            ckvg[:, k, :], start=(k == 0), stop=(k == CC - 1),
                        )
                for hh in range(4):
                    nc.scalar.copy(
                        kT[0:64, hh, gs],
                        ps[64 * (hh % 2):64 * (hh % 2) + 64, hh // 2, :])
                rope_psum(kx[:, 0, :], kx[:, 1, :], kT, gs, gs)

                for tp in range(2):
                    ps = p_psA.tile([128, 2, 512], f32, tag="ps")
                    for j in range(2):
                        tt = 2 * tp + j
                        for k in range(CC):
                            nc.tensor.matmul(
                                ps[:, j, :], ckvg[:, k, tt * 128:(tt + 1) * 128],
                                wuv[:, k, :], start=(k == 0), stop=(k == CC - 1),
                            )
                    nc.scalar.copy(v_sb[:, g * 4 + 2 * tp:g * 4 + 2 * tp + 2, :],
                                   ps[:])

                # ---- O(g-1): previous chunk's out-projection ----
                if carryO is not None:
                    cog, cattn = carryO
                    emit_O(cog, cattn)
                    carryO = None

                # ---- A(g): attention for query chunk g ----
                # kt order: diagonal (masked) tiles first so their longer
                # exp->mask chain hides under the unmasked tiles' stream.
                # Z accumulates in two tiles (vector even / pool odd) to halve
                # the serial add-chain; the normalize chain of head h is
                # issued inside head h+1's score stream.
                attn_g = p_att.tile([128, HPC, 512], f16, tag="attn")
                nkt = 4 * (g + 1)
                nktp = nkt // 2
                LOOKP = 2
                # kt pairs, diagonal (masked) pairs first
                ktp_order = ([(4 * g, 0), (4 * g + 2, 2)]
                             + [(2 * i, -1) for i in range(2 * g)])
                zchain = None

                zs1 = min(4, nktp)
                zs2 = min(6, nktp + LOOKP - 1)
                for h in range(HPC):
                    po = p_po.tile([128, 512], f32, tag="po")
                    za_v = za_p = None
                    if nktp > 2:
                        za_v = p_za.tile([128, 2, 512], f32, tag="za_v")
                        za_p = p_za.tile([128, 2, 512], f32, tag="za_p")
                    ets = {}
                    zinit = {}
                    for step in range(nktp + LOOKP):
                        if step < nktp:
                            kt0, d = ktp_order[step]
                            ps = p_psA.tile([128, 2, 512], f32, tag="ps")
                            for j in range(2):
                                nc.tensor.matmul(
                                    ps[:, j, :],
                                    kT[:, h, (kt0 + j) * 128:(kt0 + j + 1) * 128],
                                    qTg[:, h, :], start=True, stop=True,
                                )
                            et = p_et.tile([128, 2, 512], f16, tag="et")
                            nc.scalar.activation(
                                et[:], ps[:],
                                mybir.ActivationFunctionType.Exp, scale=SCALE)
                            if d >= 0:
                                meng = nc.vector if d == 0 else nc.gpsimd
                                meng.tensor_mul(et[:], et[:],
                                                masks[:, d:d + 2, :])
                            za, zeng = ((za_v, nc.vector) if step % 2 == 0
                                        else (za_p, nc.gpsimd))
                            if step < 2:
                                zinit[step % 2] = et
                            elif step < 4 and nktp > 2:
                                zeng.tensor_add(za[:], zinit.pop(step % 2)[:],
                                                et[:])
                            else:
                                zeng.tensor_add(za[:], za[:], et[:])
                            ets[step] = et
                        # deferred Z chain of previous head, issued deep into
                        # this head's stream so its za drain stays off the PE
                        # critical path
                        if step == zs1 and zchain is not None:
                            ph, pzav, pzap, ppo = zchain
                            prz = z_part1(pzav, pzap)
                            zchain = (ph, prz, ppo, True)
                        if step == zs2 and zchain is not None:
                            ph, prz, ppo, _ = zchain
                            z_part2(ph, prz, ppo, attn_g)
                            zchain = None
                        if step >= LOOKP:
                            kt0, _ = ktp_order[step - LOOKP]
                            et = ets.pop(step - LOOKP)
                            for j in range(2):
                                nc.tensor.matmul(
                                    po[:], v_sb[:, kt0 + j,
                                                h * 128:(h + 1) * 128],
                                    et[:, j, :],
                                    start=(step == LOOKP and j == 0),
                                    stop=(step == nktp + LOOKP - 1 and j == 1),
                                )
                    zav, zap = ((za_v, za_p) if nktp > 2
                                else (zinit.pop(0), zinit.pop(1)))
                    if h < HPC - 1:
                        zchain = (h, zav, zap, po)
                    else:
                        # carry last head's Z chain into the next chunk's S1
                        carryZ = (h, zav, zap, po, attn_g)
                carryO = (g, attn_g)

            # ---- epilogue: flush the last chunk's Z chain + out-projection
            ch, czav, czap, cpo, cattn = carryZ
            crz = z_part1(czav, czap)
            z_part2(ch, crz, cpo, cattn)
            cog, cattn2 = carryO
            emit_O(cog, cattn2)

    nc.compile()
    return nc


# ================= host-side prep =================

def _rope_tables(S):
    inv_freq = 1.0 / (ROPE_BASE ** (np.arange(HALF, dtype=np.float64) / HALF))
    ang = np.arange(S, dtype=np.float64)[:, None] * inv_freq[None, :]   # [S, 32]
    cosA = np.tile(np.cos(ang).T, (4, 1)).astype(np.float16)           # [128, S]
    sinA = np.tile(np.sin(ang).T, (4, 1)).astype(np.float16)
    return cosA, sinA


def _masks01():
    p = np.arange(128)[:, None]
    j = np.arange(512)[None, :]
    m = np.zeros((128, 4, 512), dtype=np.float16)
    for d in range(4):
        m[:, d, :] = (d * 128 + p <= j).astype(np.float16)
    return m


def _core_inputs(core, x, W_dq, W_dkv, W_uq, W_uk, W_uv, W_kr, W_qr, W_o, S):
    b, hg = core // 4, core % 4
    h0 = hg * HPC

    def pm(w):  # [R, C] -> [128, R//128, C] partition-major
        R, Cc = w.shape
        return np.ascontiguousarray(
            w.reshape(R // 128, 128, Cc).transpose(1, 0, 2)).astype(np.float16)

    heads = np.arange(h0, h0 + HPC)
    rope_x1 = (heads[:, None] * D_ROPE + np.arange(HALF)[None, :]).reshape(-1)
    rope_x2 = rope_x1 + HALF
    nope_cols = (heads[:, None] * D_NOPE + np.arange(D_NOPE)[None, :]).reshape(-1)
    v_cols = (heads[:, None] * D_HEAD + np.arange(D_HEAD)[None, :]).reshape(-1)

    # fold W_dq @ [W_uq | W_qr] -> direct q weights [2048, 512]
    wq_cols = np.concatenate(
        [W_uq[:, nope_cols], W_qr[:, rope_x1], W_qr[:, rope_x2]], axis=1)
    wq = (W_dq.astype(np.float64) @ wq_cols.astype(np.float64)).astype(np.float32)

    xT = np.ascontiguousarray(x[b].T)                     # [2048, S]
    TCn = S // 512
    cosA, sinA = _rope_tables(S)
    return {
        "xT": np.ascontiguousarray(
            pm(xT).reshape(128, KC, TCn, 512).transpose(2, 0, 1, 3)
        ).reshape(TCn, 128, KC * 512),
        "wq": pm(wq),
        "wdkv": pm(W_dkv),
        "wkrx1": pm(W_kr[:, rope_x1]),
        "wkrx2": pm(W_kr[:, rope_x2]),
        "wuk": pm(W_uk[:, nope_cols]),
        "wuv": pm(W_uv[:, v_cols]),
        "wo": pm(W_o[h0 * D_HEAD:(h0 + HPC) * D_HEAD, :]),
        "cosA": cosA,
        "sinA": sinA,
        "masks": _masks01(),
        "onec": np.ones((128, 1), np.float16),
        "oner": np.ones((1, 128), np.float16),
    }


_NC_CACHE = {}


def _get_nc(S):
    if S not in _NC_CACHE:
        _NC_CACHE[S] = build_nc(S)
    return _NC_CACHE[S]


def make_in_maps(inputs, S):
    args = (np.asarray(inputs["x"], np.float32),
            np.asarray(inputs["W_dq"], np.float32),
            np.asarray(inputs["W_dkv"], np.float32),
            np.asarray(inputs["W_uq"], np.float32),
            np.asarray(inputs["W_uk"], np.float32),
            np.asarray(inputs["W_uv"], np.float32),
            np.asarray(inputs["W_kr"], np.float32),
            np.asarray(inputs["W_qr"], np.float32),
            np.asarray(inputs["W_o"], np.float32))
    x, W_dq, W_dkv, W_uq, W_uk, W_uv, W_kr, W_qr, W_o = args
    return [
        _core_inputs(c, x, W_dq, W_dkv, W_uq, W_uk, W_uv, W_kr, W_qr, W_o, S)
        for c in range(8)
    ]


def kernel(x, W_dkv, W_dq, W_uq, W_uk, W_uv, W_kr, W_qr, W_o, _trace=False):
    S = x.shape[1]
    nc = _get_nc(S)
    in_maps = make_in_maps(dict(x=x, W_dq=W_dq, W_dkv=W_dkv, W_uq=W_uq,
                                W_uk=W_uk, W_uv=W_uv, W_kr=W_kr, W_qr=W_qr,
                                W_o=W_o), S)
    res = bass_utils.run_bass_kernel_spmd(nc, in_maps, core_ids=list(range(8)),
                                          trace=_trace)
    out = np.zeros((B, S, H_DIM), np.float32)
    for c in range(8):
        out[c // 4] += res.results[c]["o"]
    if _trace:
        kernel.last_exec_time_ns = res.exec_time_ns
        kernel.last_results = res
    return out


# revision 39
# speedup vs baseline: 1.1886x; 1.0061x over previous
"""MLA (multi-head latent attention) Trainium2 Bass kernel, 8-core SPMD.

Sharding: 2-way data parallel over batch x 4-way tensor parallel over heads.
Core c handles batch b = c // 4 and heads [hg*4, hg*4+4) with hg = c % 4.
Each core computes the full MLA forward for its batch/heads and the partial
out-projection (row-sharded W_o); the host sums the 4 partials per batch.

Fully fused per-chunk pipeline. For each 512-token chunk g:
  S1: q (folded W_dq@[W_uq|W_qr], direct from x), c_kv, krx chains
  S2: k up-proj + rope, v up-proj
  O(g-1): previous chunk's out-projection
  A(g): causal attention for query chunk g over key chunks 0..g
All PSUM work uses 2-bank [128,2,512] pair tiles: two matmul chains per
tile, one exp / one eviction per pair (halves the per-instruction
overhead). Engine balance: PE does all matmuls; Scalar does exp + most
PSUM evictions; Vector and Pool (gpsimd) split the causal 0/1 post-exp
mask muls and the dual softmax-Z accumulators; Vector also does rope and
the final 1/Z normalize mul. Score pairs are issued 2 ahead of the AV
matmuls; each head's Z-normalize chain is deferred deep into the next
head's score stream (the last head's into the next chunk's S1, as is the
out-projection) so the PE never waits on exp/Z. DMA queues are
dedicated: x chunks on the scalar queue, weights in first-use order on
the sync queue, small/late tensors on the gpsimd SWDGE queue.

Layouts on device (partition dim first):
  xT      [128, 16, S]   x[b].T, feature-on-partition
  qTg     [128, 4, 512]  per head: rows 0:64 content, 64:96/96:128 rope halves
  kT      [128, 4, S]    same row layout
  v       [128, 16, 512] [token%128, token//128, head*128+d]
  scoresT [128k, 2, 512q] PSUM pair; exp'd on Scalar -> et (fp16); Z via
                         v/p accumulators + ones-matmul; normalize (1/Z
                         broadcast via ones-matmul) at attn eviction.
All matmul operands are fp16 (1 cyc/row at full PE clock).
"""
import sys

sys.path.insert(0, "/opt/trn_rl_repo")

import numpy as np

import concourse.bacc as bacc
import concourse.mybir as mybir
import concourse.tile as tile
from concourse import bass_utils

H_DIM = 2048
N_HEADS = 16
D_HEAD = 128
D_ROPE = 64
D_NOPE = 64
HALF = D_ROPE // 2          # 32
C_DIM = 512
ROPE_BASE = 10000.0
HPC = 4                     # heads per core
B = 2
S_FULL = 2048
KC = H_DIM // 128           # 16
CC = C_DIM // 128           # 4
SCALE = 1.0 / float(np.sqrt(D_HEAD))

f16 = mybir.dt.float16
f32 = mybir.dt.float32


def build_nc(S=S_FULL):
    TC = S // 512            # token chunks / query groups

    nc = bacc.Bacc("TRN2", target_bir_lowering=False, debug=False)

    d_xT = nc.dram_tensor("xT", [TC, 128, KC * 512], f16, kind="ExternalInput")
    d_wq = nc.dram_tensor("wq", [128, KC, 512], f16, kind="ExternalInput")
    d_wdkv = nc.dram_tensor("wdkv", [128, KC, C_DIM], f16, kind="ExternalInput")
    d_wkrx1 = nc.dram_tensor("wkrx1", [128, KC, HPC * HALF], f16, kind="ExternalInput")
    d_wkrx2 = nc.dram_tensor("wkrx2", [128, KC, HPC * HALF], f16, kind="ExternalInput")
    d_wuk = nc.dram_tensor("wuk", [128, CC, HPC * D_NOPE], f16, kind="ExternalInput")
    d_wuv = nc.dram_tensor("wuv", [128, CC, HPC * D_HEAD], f16, kind="ExternalInput")
    d_wo = nc.dram_tensor("wo", [128, HPC, H_DIM], f16, kind="ExternalInput")
    d_cos = nc.dram_tensor("cosA", [128, S], f16, kind="ExternalInput")
    d_sin = nc.dram_tensor("sinA", [128, S], f16, kind="ExternalInput")
    d_mask = nc.dram_tensor("masks", [128, 4, 512], f16, kind="ExternalInput")
    d_onec = nc.dram_tensor("onec", [128, 1], f16, kind="ExternalInput")
    d_oner = nc.dram_tensor("oner", [1, 128], f16, kind="ExternalInput")
    d_o = nc.dram_tensor("o", [S, H_DIM], f32, kind="ExternalOutput")

    import contextlib
    with tile.TileContext(nc) as tc:
        with contextlib.ExitStack() as stack:
            def pool(name, **kw):
                return stack.enter_context(tc.tile_pool(name=name, **kw))

            p_const = pool("const", bufs=1)
            p_w = pool("w", bufs=1, side="right")
            p_x = pool("xp", bufs=2, side="right")
            p_kT = pool("kT", bufs=1)
            p_v = pool("vp", bufs=1)
            p_qT = pool("qT", bufs=2)
            p_ckv = pool("ckv", bufs=2, side="right")
            p_krx = pool("krx", bufs=1, side="right")
            p_tmp = pool("tmp", bufs=1, side="right")
            p_et = pool("et", bufs=4)
            p_za = pool("za", bufs=2)
            p_rz = pool("rz", bufs=1)
            p_rb = pool("rb", bufs=1)
            p_att = pool("att", bufs=2)
            p_ot = pool("ot", bufs=2)
            p_psA = pool("psA", bufs=2, space="PSUM")
            p_po = pool("ps_po", bufs=2, space="PSUM")
            p_pz = pool("ps_z", bufs=1, space="PSUM")
            p_pb = pool("ps_b", bufs=1, space="PSUM")

            # ---- constants + weights, ordered by first use. x chunks go on
            # the scalar engine's DMA queue so they overlap the weight DMAs
            # (sync queue).
            # Queue plan: scalar queue carries ONLY x chunks (chunk 0 first,
            # nothing ahead of it); sync queue carries the weights in first-use
            # order; the gpsimd SWDGE queue carries the small / late tensors.
            wq = p_w.tile([128, KC, 512], f16, tag="wq")
            nc.sync.dma_start(wq[:], d_wq.ap())
            wdkv = p_w.tile([128, KC, C_DIM], f16, tag="wdkv")
            nc.sync.dma_start(wdkv[:], d_wdkv.ap())
            wkrx1 = p_w.tile([128, KC, HPC * HALF], f16, tag="wkrx1")
            nc.sync.dma_start(wkrx1[:], d_wkrx1.ap())
            wkrx2 = p_w.tile([128, KC, HPC * HALF], f16, tag="wkrx2")
            nc.sync.dma_start(wkrx2[:], d_wkrx2.ap())
            wo = p_w.tile([128, HPC, H_DIM], f16, tag="wo")
            nc.sync.dma_start(wo[:], d_wo.ap())
            cosA = p_const.tile([128, S], f16, tag="cosA")
            nc.gpsimd.dma_start(cosA[:], d_cos.ap())
            sinA = p_const.tile([128, S], f16, tag="sinA")
            nc.gpsimd.dma_start(sinA[:], d_sin.ap())
            wuk = p_w.tile([128, CC, HPC * D_NOPE], f16, tag="wuk")
            nc.gpsimd.dma_start(wuk[:], d_wuk.ap())
            wuv = p_w.tile([128, CC, HPC * D_HEAD], f16, tag="wuv")
            nc.gpsimd.dma_start(wuv[:], d_wuv.ap())
            masks = p_const.tile([128, 4, 512], f16, tag="masks")
            nc.gpsimd.dma_start(masks[:], d_mask.ap())
            onech = p_const.tile([128, 1], f16, tag="onec")
            nc.gpsimd.dma_start(onech[:], d_onec.ap())
            oner = p_const.tile([1, 128], f16, tag="oner")
            nc.gpsimd.dma_start(oner[:], d_oner.ap())

            # persistent K/V for all chunks
            kT = p_kT.tile([128, HPC, S], f16, tag="kT")
            v_sb = p_v.tile([128, S // 128, 512], f16, tag="v")

            def rope_psum(x1ap, x2ap, dst, gs, dc):
                # x1/x2: [128 = 4h*32, 512] (psum or sbuf); writes rope rows
                t1 = p_tmp.tile([128, 512], f16, tag="t1")
                t2 = p_tmp.tile([128, 512], f16, tag="t2")
                o1 = p_tmp.tile([128, 512], f16, tag="o1")
                o2 = p_tmp.tile([128, 512], f16, tag="o2")
                nc.vector.tensor_mul(t1[:], x1ap, cosA[:, gs])
                nc.vector.tensor_mul(t2[:], x2ap, sinA[:, gs])
                nc.vector.tensor_sub(o1[:], t1[:], t2[:])
                nc.vector.tensor_mul(t1[:], x1ap, sinA[:, gs])
                nc.vector.tensor_mul(t2[:], x2ap, cosA[:, gs])
                nc.vector.tensor_add(o2[:], t1[:], t2[:])
                for h in range(HPC):
                    hs = slice(h * HALF, (h + 1) * HALF)
                    nc.scalar.copy(dst[64:96, h, dc], o1[hs, :])
                    nc.scalar.copy(dst[96:128, h, dc], o2[hs, :])

            def z_part1(za_v, za_p):
                # zr[q]: (za_v + za_p) then summed over the pair dim
                zm = p_rb.tile([128, 2, 512], f16, tag="zm")
                nc.vector.tensor_add(zm[:], za_v[:], za_p[:])
                zr = p_rb.tile([128, 512], f16, tag="zr")
                nc.vector.tensor_add(zr[:], zm[:, 0, :], zm[:, 1, :])
                pz = p_pz.tile([1, 512], f32, tag="pz")
                nc.tensor.matmul(pz[:], onech[:], zr[:], start=True,
                                 stop=True)
                r0 = p_rz.tile([1, 512], f32, tag="r0")
                nc.vector.reciprocal(r0[:], pz[:])
                rzc = p_rz.tile([1, 512], f16, tag="rzc")
                nc.vector.tensor_copy(rzc[:], r0[:])
                return rzc

            def z_part2(h, rzc, po, attn_t):
                pb = p_pb.tile([128, 512], f32, tag="pb")
                nc.tensor.matmul(pb[:], oner[:], rzc[:], start=True, stop=True)
                rb = p_rb.tile([128, 512], f16, tag="rb")
                nc.scalar.copy(rb[:], pb[:])
                nc.vector.tensor_mul(attn_t[:, h, :], po[:], rb[:])

            def emit_O(og, attn_t):
                # out-projection for chunk og; nck pairs share one 2-bank
                # PSUM tile, evicted with a single copy + single DMA
                for t4 in range(4):
                    tt = og * 4 + t4
                    for np2 in range(2):
                        pso = p_psA.tile([128, 2, 512], f32, tag="ps")
                        for j in range(2):
                            nck = 2 * np2 + j
                            for h in range(HPC):
                                nc.tensor.matmul(
                                    pso[:, j, :],
                                    attn_t[:, h, t4 * 128:(t4 + 1) * 128],
                                    wo[:, h, nck * 512:(nck + 1) * 512],
                                    start=(h == 0), stop=(h == HPC - 1),
                                )
                        ot = p_ot.tile([128, 2, 512], f32, tag="ot")
                        if (t4 + np2) % 2 == 0:
                            nc.vector.tensor_copy(ot[:], pso[:])
                        else:
                            nc.scalar.copy(ot[:], pso[:])
                        nc.sync.dma_start(
                            d_o.ap()[tt * 128:(tt + 1) * 128,
                                     np2 * 1024:(np2 + 1) * 1024],
                            ot[:])

            carryZ = None   # last head of previous chunk: (h, za_v, za_p, po, attn)
            carryO = None   # previous chunk's out-projection: (g, attn)

            for g in range(TC):
                gs = slice(g * 512, (g + 1) * 512)
                xt = p_x.tile([128, KC, 512], f16, tag="xt")
                nc.scalar.dma_start(xt[:], d_xT.ap()[g])

                # ---- S1a: q direct from x (folded weights) ----
                # The previous chunk's last-head Z chain is issued between the
                # first chains so the PE never waits on it.
                qTg = p_qT.tile([128, HPC, 512], f16, tag="qTg")
                qrope = None
                for mp in range(2):
                    ps = p_psA.tile([128, 2, 512], f32, tag="ps")
                    for j in range(2):
                        m = 2 * mp + j
                        for k in range(KC):
                            nc.tensor.matmul(
                                ps[:, j, :], wq[:, k, m * 128:(m + 1) * 128],
                                xt[:, k, :],
                                start=(k == 0), stop=(k == KC - 1),
                            )
                    if mp == 0:
                        for hh in range(4):
                            nc.scalar.copy(
                                qTg[0:64, hh, :],
                                ps[64 * (hh % 2):64 * (hh % 2) + 64, hh // 2, :])
                        if carryZ is not None:
                            ch, czav, czap, cpo, cattn = carryZ
                            crz = z_part1(czav, czap)
                            carryZ = (ch, crz, cpo, cattn)
                    else:
                        qrope = ps
                        if carryZ is not None:
                            ch, crz, cpo, cattn = carryZ
                            z_part2(ch, crz, cpo, cattn)
                            carryZ = None
                rope_psum(qrope[:, 0, :], qrope[:, 1, :], qTg, gs, slice(0, 512))

                # ---- S1b: c_kv chains ----
                ckvg = p_ckv.tile([128, CC, 512], f16, tag="ckvg")
                for mp in range(2):
                    ps = p_psA.tile([128, 2, 512], f32, tag="ps")
                    for j in range(2):
                        m = 2 * mp + j
                        for k in range(KC):
                            nc.tensor.matmul(
                                ps[:, j, :], wdkv[:, k, m * 128:(m + 1) * 128],
                                xt[:, k, :],
                                start=(k == 0), stop=(k == KC - 1),
                            )
                    nc.scalar.copy(ckvg[:, 2 * mp:2 * mp + 2, :], ps[:])

                # ---- S1c: krx chains ----
                kx = p_krx.tile([128, 2, 512], f16, tag="kx")
                ps = p_psA.tile([128, 2, 512], f32, tag="ps")
                for j, w_sb in enumerate((wkrx1, wkrx2)):
                    for k in range(KC):
                        nc.tensor.matmul(
                            ps[:, j, :], w_sb[:, k, :], xt[:, k, :],
                            start=(k == 0), stop=(k == KC - 1),
                        )
                nc.scalar.copy(kx[:], ps[:])

                # ---- S2: k up-proj (content) + rope; v up-proj ----
                ps = p_psA.tile([128, 2, 512], f32, tag="ps")
                for m2 in range(2):
                    for k in range(CC):
                        nc.tensor.matmul(
                            ps[:, m2, :], wuk[:, k, m2 * 128:(m2 + 1) * 128],
                            ckvg[:, k, :], start=(k == 0), stop=(k == CC - 1),
                        )
                for hh in range(4):
                    nc.scalar.copy(
                        kT[0:64, hh, gs],
                        ps[64 * (hh % 2):64 * (hh % 2) + 64, hh // 2, :])
                rope_psum(kx[:, 0, :], kx[:, 1, :], kT, gs, gs)

                for tp in range(2):
                    ps = p_psA.tile([128, 2, 512], f32, tag="ps")
                    for j in range(2):
                        tt = 2 * tp + j
                        for k in range(CC):
                            nc.tensor.matmul(
                                ps[:, j, :], ckvg[:, k, tt * 128:(tt + 1) * 128],
                                wuv[:, k, :], start=(k == 0), stop=(k == CC - 1),
                            )
                    nc.scalar.copy(v_sb[:, g * 4 + 2 * tp:g * 4 + 2 * tp + 2, :],
                                   ps[:])

                # ---- O(g-1): previous chunk's out-projection ----
                if carryO is not None:
                    cog, cattn = carryO
                    emit_O(cog, cattn)
                    carryO = None

                # ---- A(g): attention for query chunk g ----
                # kt order: diagonal (masked) tiles first so their longer
                # exp->mask chain hides under the unmasked tiles' stream.
                # Z accumulates in two tiles (vector even / pool odd) to halve
                # the serial add-chain; the normalize chain of head h is
                # issued inside head h+1's score stream.
                attn_g = p_att.tile([128, HPC, 512], f16, tag="attn")
                nkt = 4 * (g + 1)
                nktp = nkt // 2
                LOOKP = 2
                # kt pairs, diagonal (masked) pairs first
                ktp_order = ([(4 * g, 0), (4 * g + 2, 2)]
                             + [(2 * i, -1) for i in range(2 * g)])
                zchain = None

                zs1 = min(4, nktp)
                zs2 = min(6, nktp + LOOKP - 1)
                for h in range(HPC):
                    po = p_po.tile([128, 512], f32, tag="po")
                    za_v = za_p = None
                    if nktp > 2:
                        za_v = p_za.tile([128, 2, 512], f32, tag="za_v")
                        za_p = p_za.tile([128, 2, 512], f32, tag="za_p")
                    ets = {}
                    zinit = {}
                    for step in range(nktp + LOOKP):
                        if step < nktp:
                            kt0, d = ktp_order[step]
                            ps = p_psA.tile([128, 2, 512], f32, tag="ps")
                            for j in range(2):
                                nc.tensor.matmul(
                                    ps[:, j, :],
                                    kT[:, h, (kt0 + j) * 128:(kt0 + j + 1) * 128],
                                    qTg[:, h, :], start=True, stop=True,
                                )
                            et = p_et.tile([128, 2, 512], f16, tag="et")
                            nc.scalar.activation(
                                et[:], ps[:],
                                mybir.ActivationFunctionType.Exp, scale=SCALE)
                            if d >= 0:
                                meng = nc.vector if d == 0 else nc.gpsimd
                                meng.tensor_mul(et[:], et[:],
                                                masks[:, d:d + 2, :])
                            za, zeng = ((za_v, nc.vector) if step % 2 == 0
                                        else (za_p, nc.gpsimd))
                            if step < 2:
                                zinit[step % 2] = et
                            elif step < 4 and nktp > 2:
                                zeng.tensor_add(za[:], zinit.pop(step % 2)[:],
                                                et[:])
                            else:
                                zeng.tensor_add(za[:], za[:], et[:])
                            ets[step] = et
                        # deferred Z chain of previous head, issued deep into
                        # this head's stream so its za drain stays off the PE
                        # critical path
                        if step == zs1 and zchain is not None:
                            ph, pzav, pzap, ppo = zchain
                            prz = z_part1(pzav, pzap)
                            zchain = (ph, prz, ppo, True)
                        if step == zs2 and zchain is not None:
                            ph, prz, ppo, _ = zchain
                            z_part2(ph, prz, ppo, attn_g)
                            zchain = None
                        if step >= LOOKP:
                            kt0, _ = ktp_order[step - LOOKP]
                            et = ets.pop(step - LOOKP)
                            for j in range(2):
                                nc.tensor.matmul(
                                    po[:], v_sb[:, kt0 + j,
                                                h * 128:(h + 1) * 128],
                                    et[:, j, :],
                                    start=(step == LOOKP and j == 0),
                                    stop=(step == nktp + LOOKP - 1 and j == 1),
                                )
                    zav, zap = ((za_v, za_p) if nktp > 2
                                else (zinit.pop(0), zinit.pop(1)))
                    if h < HPC - 1:
                        zchain = (h, zav, zap, po)
                    else:
                        # carry last head's Z chain into the next chunk's S1
                        carryZ = (h, zav, zap, po, attn_g)
                carryO = (g, attn_g)

            # ---- epilogue: flush the last chunk's Z chain + out-projection
            ch, czav, czap, cpo, cattn = carryZ
            crz = z_part1(czav, czap)
            z_part2(ch, crz, cpo, cattn)
            cog, cattn2 = carryO
            emit_O(cog, cattn2)

    nc.compile()
    return nc


# ================= host-side prep =================

def _rope_tables(S):
    inv_freq = 1.0 / (ROPE_BASE ** (np.arange(HALF, dtype=np.float64) / HALF))
    ang = np.arange(S, dtype=np.float64)[:, None] * inv_freq[None, :]   # [S, 32]
    cosA = np.tile(np.cos(ang).T, (4, 1)).astype(np.float16)           # [128, S]
    sinA = np.tile(np.sin(ang).T, (4, 1)).astype(np.float16)
    return cosA, sinA


def _masks01():
    p = np.arange(128)[:, None]
    j = np.arange(512)[None, :]
    m = np.zeros((128, 4, 512), dtype=np.float16)
    for d in range(4):
        m[:, d, :] = (d * 128 + p <= j).astype(np.float16)
    return m


def _core_inputs(core, x, W_dq, W_dkv, W_uq, W_uk, W_uv, W_kr, W_qr, W_o, S):
    b, hg = core // 4, core % 4
    h0 = hg * HPC

    def pm(w):  # [R, C] -> [128, R//128, C] partition-major
        R, Cc = w.shape
        return np.ascontiguousarray(
            w.reshape(R // 128, 128, Cc).transpose(1, 0, 2)).astype(np.float16)

    heads = np.arange(h0, h0 + HPC)
    rope_x1 = (heads[:, None] * D_ROPE + np.arange(HALF)[None, :]).reshape(-1)
    rope_x2 = rope_x1 + HALF
    nope_cols = (heads[:, None] * D_NOPE + np.arange(D_NOPE)[None, :]).reshape(-1)
    v_cols = (heads[:, None] * D_HEAD + np.arange(D_HEAD)[None, :]).reshape(-1)

    # fold W_dq @ [W_uq | W_qr] -> direct q weights [2048, 512]
    wq_cols = np.concatenate(
        [W_uq[:, nope_cols], W_qr[:, rope_x1], W_qr[:, rope_x2]], axis=1)
    wq = (W_dq.astype(np.float64) @ wq_cols.astype(np.float64)).astype(np.float32)

    xT = np.ascontiguousarray(x[b].T)                     # [2048, S]
    TCn = S // 512
    cosA, sinA = _rope_tables(S)
    return {
        "xT": np.ascontiguousarray(
            pm(xT).reshape(128, KC, TCn, 512).transpose(2, 0, 1, 3)
        ).reshape(TCn, 128, KC * 512),
        "wq": pm(wq),
        "wdkv": pm(W_dkv),
        "wkrx1": pm(W_kr[:, rope_x1]),
        "wkrx2": pm(W_kr[:, rope_x2]),
        "wuk": pm(W_uk[:, nope_cols]),
        "wuv": pm(W_uv[:, v_cols]),
        "wo": pm(W_o[h0 * D_HEAD:(h0 + HPC) * D_HEAD, :]),
        "cosA": cosA,
        "sinA": sinA,
        "masks": _masks01(),
        "onec": np.ones((128, 1), np.float16),
        "oner": np.ones((1, 128), np.float16),
    }


_NC_CACHE = {}


def _get_nc(S):
    if S not in _NC_CACHE:
        _NC_CACHE[S] = build_nc(S)
    return _NC_CACHE[S]


def make_in_maps(inputs, S):
    args = (np.asarray(inputs["x"], np.float32),
            np.asarray(inputs["W_dq"], np.float32),
            np.asarray(inputs["W_dkv"], np.float32),
            np.asarray(inputs["W_uq"], np.float32),
            np.asarray(inputs["W_uk"], np.float32),
            np.asarray(inputs["W_uv"], np.float32),
            np.asarray(inputs["W_kr"], np.float32),
            np.asarray(inputs["W_qr"], np.float32),
            np.asarray(inputs["W_o"], np.float32))
    x, W_dq, W_dkv, W_uq, W_uk, W_uv, W_kr, W_qr, W_o = args
    return [
        _core_inputs(c, x, W_dq, W_dkv, W_uq, W_uk, W_uv, W_kr, W_qr, W_o, S)
        for c in range(8)
    ]


def kernel(x, W_dkv, W_dq, W_uq, W_uk, W_uv, W_kr, W_qr, W_o, _trace=False):
    S = x.shape[1]
    nc = _get_nc(S)
    in_maps = make_in_maps(dict(x=x, W_dq=W_dq, W_dkv=W_dkv, W_uq=W_uq,
                                W_uk=W_uk, W_uv=W_uv, W_kr=W_kr, W_qr=W_qr,
                                W_o=W_o), S)
    res = bass_utils.run_bass_kernel_spmd(nc, in_maps, core_ids=list(range(8)),
                                          trace=_trace)
    out = np.zeros((B, S, H_DIM), np.float32)
    for c in range(8):
        out[c // 4] += res.results[c]["o"]
    if _trace:
        kernel.last_exec_time_ns = res.exec_time_ns
        kernel.last_results = res
    return out


# revision 40
# speedup vs baseline: 1.2266x; 1.0320x over previous
"""MLA (multi-head latent attention) Trainium2 Bass kernel, 8-core SPMD.

Sharding: 2-way data parallel over batch x 4-way tensor parallel over heads.
Core c handles batch b = c // 4 and heads [hg*4, hg*4+4) with hg = c % 4.
Each core computes the full MLA forward for its batch/heads and the partial
out-projection (row-sharded W_o); the host sums the 4 partials per batch.

Fully fused per-chunk pipeline. For each 512-token chunk g:
  S1: q (folded W_dq@[W_uq|W_qr], direct from x), c_kv, krx chains
  S2: k up-proj + rope, v up-proj
  O(g-1): previous chunk's out-projection
  A(g): causal attention for query chunk g over key chunks 0..g
All PSUM work uses 2-bank [128,2,512] pair tiles: two matmul chains per
tile, one exp / one eviction per pair (halves the per-instruction
overhead). Engine balance: PE does all matmuls; Scalar does exp + most
PSUM evictions; Vector and Pool (gpsimd) split the causal 0/1 post-exp
mask muls and the dual softmax-Z accumulators; Vector also does rope and
the final 1/Z normalize mul. Score pairs are issued 2 ahead of the AV
matmuls; each head's Z-normalize chain is deferred deep into the next
head's score stream (the last head's into the next chunk's S1, as is the
out-projection) so the PE never waits on exp/Z. DMA queues are
dedicated: x chunks on the scalar queue, weights in first-use order on
the sync queue, small/late tensors on the gpsimd SWDGE queue.

Layouts on device (partition dim first):
  xT      [128, 16, S]   x[b].T, feature-on-partition
  qTg     [128, 4, 512]  per head: rows 0:64 content, 64:96/96:128 rope halves
  kT      [128, 4, S]    same row layout
  v       [128, 16, 512] [token%128, token//128, head*128+d]
  scoresT [128k, 2, 512q] PSUM pair; exp'd on Scalar -> et (fp16); Z via
                         v/p accumulators + ones-matmul; normalize (1/Z
                         broadcast via ones-matmul) at attn eviction.
All matmul operands are fp16 (1 cyc/row at full PE clock).
"""
import sys

sys.path.insert(0, "/opt/trn_rl_repo")

import numpy as np

import concourse.bacc as bacc
import concourse.mybir as mybir
import concourse.tile as tile
from concourse import bass_utils

H_DIM = 2048
N_HEADS = 16
D_HEAD = 128
D_ROPE = 64
D_NOPE = 64
HALF = D_ROPE // 2          # 32
C_DIM = 512
ROPE_BASE = 10000.0
HPC = 4                     # heads per core
B = 2
S_FULL = 2048
KC = H_DIM // 128           # 16
CC = C_DIM // 128           # 4
SCALE = 1.0 / float(np.sqrt(D_HEAD))

f16 = mybir.dt.float16
f32 = mybir.dt.float32


def build_nc(S=S_FULL):
    TC = S // 512            # token chunks / query groups

    nc = bacc.Bacc("TRN2", target_bir_lowering=False, debug=False)

    d_xT = nc.dram_tensor("xT", [TC, 128, KC * 512], f16, kind="ExternalInput")
    d_wq = nc.dram_tensor("wq", [128, KC, 512], f16, kind="ExternalInput")
    d_wdkv = nc.dram_tensor("wdkv", [128, KC, C_DIM], f16, kind="ExternalInput")
    d_wkrx1 = nc.dram_tensor("wkrx1", [128, KC, HPC * HALF], f16, kind="ExternalInput")
    d_wkrx2 = nc.dram_tensor("wkrx2", [128, KC, HPC * HALF], f16, kind="ExternalInput")
    d_wuk = nc.dram_tensor("wuk", [128, CC, HPC * D_NOPE], f16, kind="ExternalInput")
    d_wuv = nc.dram_tensor("wuv", [128, CC, HPC * D_HEAD], f16, kind="ExternalInput")
    d_wo = nc.dram_tensor("wo", [128, HPC, H_DIM], f16, kind="ExternalInput")
    d_cos = nc.dram_tensor("cosA", [128, S], f16, kind="ExternalInput")
    d_sin = nc.dram_tensor("sinA", [128, S], f16, kind="ExternalInput")
    d_mask = nc.dram_tensor("masks", [128, 4, 512], f16, kind="ExternalInput")
    d_onec = nc.dram_tensor("onec", [128, 1], f16, kind="ExternalInput")
    d_oner = nc.dram_tensor("oner", [1, 128], f16, kind="ExternalInput")
    d_o = nc.dram_tensor("o", [S, H_DIM], f32, kind="ExternalOutput")

    import contextlib
    with tile.TileContext(nc) as tc:
        with contextlib.ExitStack() as stack:
            def pool(name, **kw):
                return stack.enter_context(tc.tile_pool(name=name, **kw))

            p_const = pool("const", bufs=1)
            p_w = pool("w", bufs=1, side="right")
            p_x = pool("xp", bufs=2, side="right")
            p_kT = pool("kT", bufs=1)
            p_v = pool("vp", bufs=1)
            p_qT = pool("qT", bufs=2)
            p_ckv = pool("ckv", bufs=2, side="right")
            p_krx = pool("krx", bufs=1, side="right")
            p_tmp = pool("tmp", bufs=1, side="right")
            p_et = pool("et", bufs=4)
            p_za = pool("za", bufs=2)
            p_rz = pool("rz", bufs=1)
            p_rb = pool("rb", bufs=1)
            p_att = pool("att", bufs=2)
            p_ot = pool("ot", bufs=2)
            p_psA = pool("psA", bufs=2, space="PSUM")
            p_po = pool("ps_po", bufs=2, space="PSUM")
            p_pz = pool("ps_z", bufs=1, space="PSUM")
            p_pb = pool("ps_b", bufs=1, space="PSUM")

            # ---- constants + weights, ordered by first use. x chunks go on
            # the scalar engine's DMA queue so they overlap the weight DMAs
            # (sync queue).
            # Queue plan: scalar queue carries ONLY x chunks (chunk 0 first,
            # nothing ahead of it); sync queue carries the weights in first-use
            # order; the gpsimd SWDGE queue carries the small / late tensors.
            wq = p_w.tile([128, KC, 512], f16, tag="wq")
            nc.sync.dma_start(wq[:], d_wq.ap())
            wdkv = p_w.tile([128, KC, C_DIM], f16, tag="wdkv")
            nc.sync.dma_start(wdkv[:], d_wdkv.ap())
            wkrx1 = p_w.tile([128, KC, HPC * HALF], f16, tag="wkrx1")
            nc.sync.dma_start(wkrx1[:], d_wkrx1.ap())
            wkrx2 = p_w.tile([128, KC, HPC * HALF], f16, tag="wkrx2")
            nc.sync.dma_start(wkrx2[:], d_wkrx2.ap())
            wo = p_w.tile([128, HPC, H_DIM], f16, tag="wo")
            nc.sync.dma_start(wo[:], d_wo.ap())
            cosA = p_const.tile([128, S], f16, tag="cosA")
            nc.gpsimd.dma_start(cosA[:], d_cos.ap())
            sinA = p_const.tile([128, S], f16, tag="sinA")
            nc.gpsimd.dma_start(sinA[:], d_sin.ap())
            wuk = p_w.tile([128, CC, HPC * D_NOPE], f16, tag="wuk")
            nc.gpsimd.dma_start(wuk[:], d_wuk.ap())
            wuv = p_w.tile([128, CC, HPC * D_HEAD], f16, tag="wuv")
            nc.gpsimd.dma_start(wuv[:], d_wuv.ap())
            masks = p_const.tile([128, 4, 512], f16, tag="masks")
            nc.gpsimd.dma_start(masks[:], d_mask.ap())
            onech = p_const.tile([128, 1], f16, tag="onec")
            nc.gpsimd.dma_start(onech[:], d_onec.ap())
            oner = p_const.tile([1, 128], f16, tag="oner")
            nc.gpsimd.dma_start(oner[:], d_oner.ap())

            # persistent K/V for all chunks
            kT = p_kT.tile([128, HPC, S], f16, tag="kT")
            v_sb = p_v.tile([128, S // 128, 512], f16, tag="v")

            def rope_psum(x1ap, x2ap, dst, gs, dc):
                # x1/x2: [128 = 4h*32, 512] (psum or sbuf); writes rope rows
                t1 = p_tmp.tile([128, 512], f16, tag="t1")
                t2 = p_tmp.tile([128, 512], f16, tag="t2")
                o1 = p_tmp.tile([128, 512], f16, tag="o1")
                o2 = p_tmp.tile([128, 512], f16, tag="o2")
                nc.vector.tensor_mul(t1[:], x1ap, cosA[:, gs])
                nc.vector.tensor_mul(t2[:], x2ap, sinA[:, gs])
                nc.vector.tensor_sub(o1[:], t1[:], t2[:])
                nc.vector.tensor_mul(t1[:], x1ap, sinA[:, gs])
                nc.vector.tensor_mul(t2[:], x2ap, cosA[:, gs])
                nc.vector.tensor_add(o2[:], t1[:], t2[:])
                for h in range(HPC):
                    hs = slice(h * HALF, (h + 1) * HALF)
                    nc.scalar.copy(dst[64:96, h, dc], o1[hs, :])
                    nc.scalar.copy(dst[96:128, h, dc], o2[hs, :])

            def z_part1(za_v, za_p):
                # zr[q]: (za_v + za_p) then summed over the pair dim
                zm = p_rb.tile([128, 2, 512], f16, tag="zm")
                nc.vector.tensor_add(zm[:], za_v[:], za_p[:])
                zr = p_rb.tile([128, 512], f16, tag="zr")
                nc.vector.tensor_add(zr[:], zm[:, 0, :], zm[:, 1, :])
                pz = p_pz.tile([1, 512], f32, tag="pz")
                nc.tensor.matmul(pz[:], onech[:], zr[:], start=True,
                                 stop=True)
                r0 = p_rz.tile([1, 512], f32, tag="r0")
                nc.vector.reciprocal(r0[:], pz[:])
                rzc = p_rz.tile([1, 512], f16, tag="rzc")
                nc.vector.tensor_copy(rzc[:], r0[:])
                return rzc

            def z_part2(h, rzc, po, attn_t):
                pb = p_pb.tile([128, 512], f32, tag="pb")
                nc.tensor.matmul(pb[:], oner[:], rzc[:], start=True, stop=True)
                rb = p_rb.tile([128, 512], f16, tag="rb")
                nc.scalar.copy(rb[:], pb[:])
                nc.vector.tensor_mul(attn_t[:, h, :], po[:], rb[:])

            def emit_O_t4(og, attn_t, t4):
                # out-projection for token subtile t4 of chunk og; nck pairs
                # share one 2-bank PSUM tile, evicted with one copy + one DMA
                tt = og * 4 + t4
                for np2 in range(2):
                    pso = p_psA.tile([128, 2, 512], f32, tag="ps")
                    for j in range(2):
                        nck = 2 * np2 + j
                        for h in range(HPC):
                            nc.tensor.matmul(
                                pso[:, j, :],
                                attn_t[:, h, t4 * 128:(t4 + 1) * 128],
                                wo[:, h, nck * 512:(nck + 1) * 512],
                                start=(h == 0), stop=(h == HPC - 1),
                            )
                    ot = p_ot.tile([128, 2, 512], f32, tag="ot")
                    if (t4 + np2) % 2 == 0:
                        nc.vector.tensor_copy(ot[:], pso[:])
                    else:
                        nc.scalar.copy(ot[:], pso[:])
                    nc.sync.dma_start(
                        d_o.ap()[tt * 128:(tt + 1) * 128,
                                 np2 * 1024:(np2 + 1) * 1024],
                        ot[:])

            def emit_O(og, attn_t):
                for t4 in range(4):
                    emit_O_t4(og, attn_t, t4)

            carryZ = None   # last head of previous chunk: (h, za_v, za_p, po, attn)
            carryO = None   # previous chunk's out-projection: (g, attn)

            for g in range(TC):
                gs = slice(g * 512, (g + 1) * 512)
                xt = p_x.tile([128, KC, 512], f16, tag="xt")
                nc.scalar.dma_start(xt[:], d_xT.ap()[g])

                # ---- S1a: q direct from x (folded weights) ----
                # The previous chunk's last-head Z chain is issued between the
                # first chains so the PE never waits on it.
                qTg = p_qT.tile([128, HPC, 512], f16, tag="qTg")
                qrope = None
                for mp in range(2):
                    ps = p_psA.tile([128, 2, 512], f32, tag="ps")
                    for j in range(2):
                        m = 2 * mp + j
                        for k in range(KC):
                            nc.tensor.matmul(
                                ps[:, j, :], wq[:, k, m * 128:(m + 1) * 128],
                                xt[:, k, :],
                                start=(k == 0), stop=(k == KC - 1),
                            )
                    if mp == 0:
                        for hh in range(4):
                            nc.scalar.copy(
                                qTg[0:64, hh, :],
                                ps[64 * (hh % 2):64 * (hh % 2) + 64, hh // 2, :])
                        if carryZ is not None:
                            ch, czav, czap, cpo, cattn = carryZ
                            crz = z_part1(czav, czap)
                            carryZ = (ch, crz, cpo, cattn)
                    else:
                        qrope = ps
                        if carryZ is not None:
                            ch, crz, cpo, cattn = carryZ
                            z_part2(ch, crz, cpo, cattn)
                            carryZ = None
                rope_psum(qrope[:, 0, :], qrope[:, 1, :], qTg, gs, slice(0, 512))

                # ---- S1b: c_kv chains ----
                ckvg = p_ckv.tile([128, CC, 512], f16, tag="ckvg")
                for mp in range(2):
                    ps = p_psA.tile([128, 2, 512], f32, tag="ps")
                    for j in range(2):
                        m = 2 * mp + j
                        for k in range(KC):
                            nc.tensor.matmul(
                                ps[:, j, :], wdkv[:, k, m * 128:(m + 1) * 128],
                                xt[:, k, :],
                                start=(k == 0), stop=(k == KC - 1),
                            )
                    nc.scalar.copy(ckvg[:, 2 * mp:2 * mp + 2, :], ps[:])

                # ---- S1c: krx chains ----
                kx = p_krx.tile([128, 2, 512], f16, tag="kx")
                ps = p_psA.tile([128, 2, 512], f32, tag="ps")
                for j, w_sb in enumerate((wkrx1, wkrx2)):
                    for k in range(KC):
                        nc.tensor.matmul(
                            ps[:, j, :], w_sb[:, k, :], xt[:, k, :],
                            start=(k == 0), stop=(k == KC - 1),
                        )
                nc.scalar.copy(kx[:], ps[:])

                # ---- S2: k up-proj (content) + rope; v up-proj ----
                ps = p_psA.tile([128, 2, 512], f32, tag="ps")
                for m2 in range(2):
                    for k in range(CC):
                        nc.tensor.matmul(
                            ps[:, m2, :], wuk[:, k, m2 * 128:(m2 + 1) * 128],
                            ckvg[:, k, :], start=(k == 0), stop=(k == CC - 1),
                        )
                for hh in range(4):
                    nc.scalar.copy(
                        kT[0:64, hh, gs],
                        ps[64 * (hh % 2):64 * (hh % 2) + 64, hh // 2, :])
                rope_psum(kx[:, 0, :], kx[:, 1, :], kT, gs, gs)

                for tp in range(2):
                    ps = p_psA.tile([128, 2, 512], f32, tag="ps")
                    for j in range(2):
                        tt = 2 * tp + j
                        for k in range(CC):
                            nc.tensor.matmul(
                                ps[:, j, :], ckvg[:, k, tt * 128:(tt + 1) * 128],
                                wuv[:, k, :], start=(k == 0), stop=(k == CC - 1),
                            )
                    nc.scalar.copy(v_sb[:, g * 4 + 2 * tp:g * 4 + 2 * tp + 2, :],
                                   ps[:])

                # ---- A(g): attention for query chunk g, with the previous
                # chunk's out-projection interleaved one t4-group per head so
                # O matmuls fill the PE bubbles left by exp latency ----
                # kt order: diagonal (masked) tiles first so their longer
                # exp->mask chain hides under the unmasked tiles' stream.
                # Z accumulates in two tiles (vector even / pool odd) to halve
                # the serial add-chain; the normalize chain of head h is
                # issued inside head h+1's score stream.
                attn_g = p_att.tile([128, HPC, 512], f16, tag="attn")
                nkt = 4 * (g + 1)
                nktp = nkt // 2
                LOOKP = 2
                # kt pairs, diagonal (masked) pairs first
                ktp_order = ([(4 * g, 0), (4 * g + 2, 2)]
                             + [(2 * i, -1) for i in range(2 * g)])
                zchain = None

                zs1 = min(4, nktp)
                zs2 = min(6, nktp + LOOKP - 1)
                for h in range(HPC):
                    po = p_po.tile([128, 512], f32, tag="po")
                    za_v = za_p = None
                    if nktp > 2:
                        za_v = p_za.tile([128, 2, 512], f32, tag="za_v")
                        za_p = p_za.tile([128, 2, 512], f32, tag="za_p")
                    ets = {}
                    zinit = {}
                    for step in range(nktp + LOOKP):
                        if step < nktp:
                            kt0, d = ktp_order[step]
                            ps = p_psA.tile([128, 2, 512], f32, tag="ps")
                            for j in range(2):
                                nc.tensor.matmul(
                                    ps[:, j, :],
                                    kT[:, h, (kt0 + j) * 128:(kt0 + j + 1) * 128],
                                    qTg[:, h, :], start=True, stop=True,
                                )
                            et = p_et.tile([128, 2, 512], f16, tag="et")
                            nc.scalar.activation(
                                et[:], ps[:],
                                mybir.ActivationFunctionType.Exp, scale=SCALE)
                            if d >= 0:
                                meng = nc.vector if d == 0 else nc.gpsimd
                                meng.tensor_mul(et[:], et[:],
                                                masks[:, d:d + 2, :])
                            za, zeng = ((za_v, nc.vector) if step % 2 == 0
                                        else (za_p, nc.gpsimd))
                            if step < 2:
                                zinit[step % 2] = et
                            elif step < 4 and nktp > 2:
                                zeng.tensor_add(za[:], zinit.pop(step % 2)[:],
                                                et[:])
                            else:
                                zeng.tensor_add(za[:], za[:], et[:])
                            ets[step] = et
                        # deferred Z chain of previous head, issued deep into
                        # this head's stream so its za drain stays off the PE
                        # critical path
                        if step == zs1 and zchain is not None:
                            ph, pzav, pzap, ppo = zchain
                            prz = z_part1(pzav, pzap)
                            zchain = (ph, prz, ppo, True)
                        if step == zs2 and zchain is not None:
                            ph, prz, ppo, _ = zchain
                            z_part2(ph, prz, ppo, attn_g)
                            zchain = None
                        if step >= LOOKP:
                            kt0, _ = ktp_order[step - LOOKP]
                            et = ets.pop(step - LOOKP)
                            for j in range(2):
                                nc.tensor.matmul(
                                    po[:], v_sb[:, kt0 + j,
                                                h * 128:(h + 1) * 128],
                                    et[:, j, :],
                                    start=(step == LOOKP and j == 0),
                                    stop=(step == nktp + LOOKP - 1 and j == 1),
                                )
                    zav, zap = ((za_v, za_p) if nktp > 2
                                else (zinit.pop(0), zinit.pop(1)))
                    if h < HPC - 1:
                        zchain = (h, zav, zap, po)
                    else:
                        # carry last head's Z chain into the next chunk's S1
                        carryZ = (h, zav, zap, po, attn_g)
                    if carryO is not None:
                        cog, cattn = carryO
                        emit_O_t4(cog, cattn, h)
                carryO = (g, attn_g)

            # ---- epilogue: flush the last chunk's Z chain + out-projection
            ch, czav, czap, cpo, cattn = carryZ
            crz = z_part1(czav, czap)
            z_part2(ch, crz, cpo, cattn)
            cog, cattn2 = carryO
            emit_O(cog, cattn2)

    nc.compile()
    return nc


# ================= host-side prep =================

def _rope_tables(S):
    inv_freq = 1.0 / (ROPE_BASE ** (np.arange(HALF, dtype=np.float64) / HALF))
    ang = np.arange(S, dtype=np.float64)[:, None] * inv_freq[None, :]   # [S, 32]
    cosA = np.tile(np.cos(ang).T, (4, 1)).astype(np.float16)           # [128, S]
    sinA = np.tile(np.sin(ang).T, (4, 1)).astype(np.float16)
    return cosA, sinA


def _masks01():
    p = np.arange(128)[:, None]
    j = np.arange(512)[None, :]
    m = np.zeros((128, 4, 512), dtype=np.float16)
    for d in range(4):
        m[:, d, :] = (d * 128 + p <= j).astype(np.float16)
    return m


def _core_inputs(core, x, W_dq, W_dkv, W_uq, W_uk, W_uv, W_kr, W_qr, W_o, S):
    b, hg = core // 4, core % 4
    h0 = hg * HPC

    def pm(w):  # [R, C] -> [128, R//128, C] partition-major
        R, Cc = w.shape
        return np.ascontiguousarray(
            w.reshape(R // 128, 128, Cc).transpose(1, 0, 2)).astype(np.float16)

    heads = np.arange(h0, h0 + HPC)
    rope_x1 = (heads[:, None] * D_ROPE + np.arange(HALF)[None, :]).reshape(-1)
    rope_x2 = rope_x1 + HALF
    nope_cols = (heads[:, None] * D_NOPE + np.arange(D_NOPE)[None, :]).reshape(-1)
    v_cols = (heads[:, None] * D_HEAD + np.arange(D_HEAD)[None, :]).reshape(-1)

    # fold W_dq @ [W_uq | W_qr] -> direct q weights [2048, 512]
    wq_cols = np.concatenate(
        [W_uq[:, nope_cols], W_qr[:, rope_x1], W_qr[:, rope_x2]], axis=1)
    wq = (W_dq.astype(np.float64) @ wq_cols.astype(np.float64)).astype(np.float32)

    xT = np.ascontiguousarray(x[b].T)                     # [2048, S]
    TCn = S // 512
    cosA, sinA = _rope_tables(S)
    return {
        "xT": np.ascontiguousarray(
            pm(xT).reshape(128, KC, TCn, 512).transpose(2, 0, 1, 3)
        ).reshape(TCn, 128, KC * 512),
        "wq": pm(wq),
        "wdkv": pm(W_dkv),
        "wkrx1": pm(W_kr[:, rope_x1]),
        "wkrx2": pm(W_kr[:, rope_x2]),
        "wuk": pm(W_uk[:, nope_cols]),
        "wuv": pm(W_uv[:, v_cols]),
        "wo": pm(W_o[h0 * D_HEAD:(h0 + HPC) * D_HEAD, :]),
        "cosA": cosA,
        "sinA": sinA,
        "masks": _masks01(),
        "onec": np.ones((128, 1), np.float16),
        "oner": np.ones((1, 128), np.float16),
    }


_NC_CACHE = {}


def _get_nc(S):
    if S not in _NC_CACHE:
        _NC_CACHE[S] = build_nc(S)
    return _NC_CACHE[S]


def make_in_maps(inputs, S):
    args = (np.asarray(inputs["x"], np.float32),
            np.asarray(inputs["W_dq"], np.float32),
            np.asarray(inputs["W_dkv"], np.float32),
            np.asarray(inputs["W_uq"], np.float32),
            np.asarray(inputs["W_uk"], np.float32),
            np.asarray(inputs["W_uv"], np.float32),
            np.asarray(inputs["W_kr"], np.float32),
            np.asarray(inputs["W_qr"], np.float32),
            np.asarray(inputs["W_o"], np.float32))
    x, W_dq, W_dkv, W_uq, W_uk, W_uv, W_kr, W_qr, W_o = args
    return [
        _core_inputs(c, x, W_dq, W_dkv, W_uq, W_uk, W_uv, W_kr, W_qr, W_o, S)
        for c in range(8)
    ]


def kernel(x, W_dkv, W_dq, W_uq, W_uk, W_uv, W_kr, W_qr, W_o, _trace=False):
    S = x.shape[1]
    nc = _get_nc(S)
    in_maps = make_in_maps(dict(x=x, W_dq=W_dq, W_dkv=W_dkv, W_uq=W_uq,
                                W_uk=W_uk, W_uv=W_uv, W_kr=W_kr, W_qr=W_qr,
                                W_o=W_o), S)
    res = bass_utils.run_bass_kernel_spmd(nc, in_maps, core_ids=list(range(8)),
                                          trace=_trace)
    out = np.zeros((B, S, H_DIM), np.float32)
    for c in range(8):
        out[c // 4] += res.results[c]["o"]
    if _trace:
        kernel.last_exec_time_ns = res.exec_time_ns
        kernel.last_results = res
    return out
